# revision 44
# baseline (speedup 1.0000x reference)
"""Depth-wise attention over block outputs (AttentionResidual) on 8 trn2 cores.

Computation (reference):
    q' = proj[min(block_idx, maxT-1)] * norm_scale
    rms[t,r]   = sqrt(mean_d(e[t,r,:]^2) + 1e-5)
    logit[t,r] = (q' . e[t,r,:]) / rms[t,r]          (masked to -1e9 for t >= n_active)
    w = softmax_t(logit);  out[r,:] = sum_t w[t,r] * e[t,r,:]

Masked entries (t >= n_active) get softmax weight exp(-1e9 - max) == 0.0 exactly
in fp32, so only the first n_active depth slices are ever read.

Sharding: flattened B*S rows are split evenly across the 8 cores (data
parallel); q' and the identity matrix are replicated. No cross-core reduction.

The graded shape runs the _build_v7 kernel (~349 us/core measured vs a
~321 us DMA floor; the staged baseline was ~579 us).  Per core it is
HBM-bound: 12 x 8 MB of entries read + 8 MB written at ~320 GB/s.  Design
points, each worth 20-100 us on hardware:
  - 256-row blocks with 2 rows per SBUF partition so every DMA descriptor
    covers 8 KB contiguous DRAM (12 x 4 KB strided chunks cost ~27 us/iter)
  - the t dim is loaded as 12 single-t 1 MB tiles into an 18-deep buffer
    pool: an e-tile stays alive until the block's last matmul consumes it
    (~20 us after the block's final load), and fine tiles + deep pool keep
    the load queue from ever stalling on slot reuse
  - ent is DECLARED float32r in DRAM (same bits as fp32) so a plain HWDGE
    load feeds 1-cyc/row f32r matmuls (fp32 is 4 cyc/row) while the stats
    path reads the same bytes as exact fp32 via bitcast; SWDGE cast loads
    would bottleneck the Pool engine on descriptor generation
  - stats interleave the two row-chains per arriving tile; the softmax
    smalls run once per block on a u-major [128, 2T] layout (max-subtract
    on VectorE since ScalarE bias is per-partition only)
  - rinv = exp(-0.5*ln(ssq/D + eps)) on ScalarE; the act-table assigner is
    pinned (per-kernel instance) to `natural_log_exp_and_others` - stock
    first-fit flip-flops Ln->natural_log / Exp->exp_and_others and inserts
    a ~2.7 us table load TWICE PER BLOCK.  (A VectorE Newton rsqrt avoids
    tables entirely but loses 65 us: VectorE is the critical engine.)
  - TensorE: out = sum_t diag(ex_t) @ e_t into 8 single-bank PSUM
    accumulators; the PSUM->SBUF evacuation (activation Copy with the
    1/sum softmax scale folded in) and the output store are software-
    pipelined one block behind the matmuls
"""

import contextlib
import os
import sys

import numpy as np

sys.path.insert(0, "/opt/trn_rl_repo")

import concourse.bacc as bacc  # noqa: E402
import concourse.bass as bass  # noqa: E402
import concourse.tile as tile  # noqa: E402
from concourse import mybir  # noqa: E402
from concourse.bass_utils import run_bass_kernel_spmd  # noqa: E402

N_CORES = 8
P = 128
NORM_EPS = 1e-5

_kernel_cache = {}


def _pin_act_tables(nc, set_name="natural_log_exp_and_others"):
    """Make this kernel's act-table assignment use one set for everything.

    The stock first-fit assignment sends Ln to `natural_log` (no exp) and
    Square/Exp to `exp_and_others` (no ln), inserting a ~2.7us table load
    twice per block.  All four funcs we use (Square/Exp/Ln/Copy) live in
    `natural_log_exp_and_others`; emptying every other entry (positions
    preserved, so emitted set ids stay valid) makes first-fit land there
    for every activation -> exactly one load for the whole program.
    Instance-level override only; no global state touched.
    """
    import types

    import bass_rust as _brs

    from concourse.hw_specs import get_activation_tables

    def patched(self):
        has_act = any(
            isinstance(i, mybir.InstActivation)
            for b in self.main_func.blocks
            for i in b.instructions
        )
        if not has_act:
            return
        tables = [
            (name, fns if name == set_name else set())
            for name, fns in get_activation_tables(self.m.arch).items()
        ]
        _brs.insert_act_table_loads(self, tables)

    nc.insert_act_table_loads = types.MethodType(patched, nc)


def _declare_io(nc, T_act, R, D, bench_rep, ent_dt=None):
    f32 = mybir.dt.float32
    u32 = mybir.dt.uint32
    ent = nc.declare_dram_parameter("ent", [T_act, R, D], ent_dt or f32,
                                    isOutput=False)
    qv = nc.declare_dram_parameter("qv", [1, D], f32, isOutput=False)
    ident = nc.declare_dram_parameter("ident", [P, P], f32, isOutput=False)
    repc = None
    if bench_rep:
        repc = nc.declare_dram_parameter("repc", [1, 1], u32, isOutput=False)
    out = nc.declare_dram_parameter("out", [R, D], f32, isOutput=True)
    return ent, qv, ident, repc, out


def _build_v2(T_act, R, D, n_gq=0, dg_eng="gpsimd", ebufs=3, mode="full",
              mm_dtype="f32r", split_load=False, bench_rep=False, rep=1):
    """v2 builder.  T_act: active depth entries; R: rows/core; D: feature dim.

    n_gq: how many of the T_act qdot reductions run on GpSimd instead of
    VectorE.  dg_eng: engine for the diag(w) broadcast build.  mm_dtype:
    'f32r' (SWDGE rounds e to fp32r on load; PE at 1 cyc/row) | 'bf16'
    (SWDGE casts e+q' to bf16; 2x DVE qdot, half SBUF) | 'f32' (exact,
    PE at 4 cyc/row).  rep: run the body rep times in a static device
    loop (rep-delta wall-clock timing).  bench_rep: dynamic variant of
    the same via a [1,1] u32 'repc' input (hangs on HW — do not use).
    """
    f32 = mybir.dt.float32
    assert R % P == 0 and D % 512 == 0
    n_tiles = R // P

    nc = bacc.Bacc()
    ent_dt = mybir.dt.float32r if mm_dtype == "f32rd" else None
    ent, qv, ident, repc, out = _declare_io(nc, T_act, R, D, bench_rep,
                                            ent_dt=ent_dt)

    with tile.TileContext(nc) as tc:
        with (
            tc.tile_pool(name="singles", bufs=1) as singles,
            tc.tile_pool(name="ebuf", bufs=ebufs) as ebuf,
            tc.tile_pool(name="stats", bufs=2) as stats,
            tc.tile_pool(name="scr", bufs=1) as scr,
            tc.tile_pool(name="diag", bufs=2) as diagp,
            tc.tile_pool(name="outs", bufs=3) as outs,
            tc.tile_pool(name="psum", bufs=2, space="PSUM") as psump,
        ):
            qb_dt = mybir.dt.bfloat16 if mm_dtype == "bf16" else f32
            qb = singles.tile([P, D], qb_dt)
            nc.gpsimd.dma_start(out=qb, in_=qv[:, :].to_broadcast((P, D)))
            id_t = singles.tile([P, P], f32)
            nc.sync.dma_start(out=id_t, in_=ident[:, :])
            epsb = singles.tile([P, 1], f32)
            nc.vector.memset(epsb, float(NORM_EPS))

            if bench_rep:
                rt = singles.tile([1, 1], mybir.dt.uint32)
                nc.sync.dma_start(out=rt, in_=repc[:, :])
                _, (repv,) = nc.values_load_multi_w_load_instructions(
                    rt[0:1, 0:1], min_val=1, max_val=1 << 20
                )
                loop_ctx = tc.For_i(0, repv, 1)
            elif rep > 1:
                loop_ctx = tc.For_i(0, rep, 1)
            else:
                loop_ctx = contextlib.nullcontext()
            with loop_ctx:
                _v2_body(nc, T_act, D, n_tiles, n_gq, dg_eng, mode, mm_dtype,
                         split_load, ent, out, qb, id_t, epsb, ebuf, stats,
                         scr, diagp, outs, psump)

    nc.finalize()
    return nc


def _v2_body(nc, T_act, D, n_tiles, n_gq, dg_eng, mode, mm_dtype, split_load,
             ent, out, qb, id_t, epsb, ebuf, stats, scr, diagp, outs, psump):
    f32 = mybir.dt.float32
    AF = mybir.ActivationFunctionType
    OP = mybir.AluOpType
    nh = D // 512
    e_dt = {"f32": f32, "f32r": mybir.dt.float32r,
            "f32rd": mybir.dt.float32r, "bf16": mybir.dt.bfloat16}[mm_dtype]
    dg_dt = e_dt if mm_dtype != "f32" else f32

    for i in range(n_tiles):
        r0 = i * P
        if mode == "dmacontig":
            # BW probe: same bytes per tile but one contiguous 48KB chunk
            # per partition instead of 12 strided 4KB chunks.  Not a valid
            # compute layout.
            base = ent[:, 0:1, :]
            flat = bass.AP(
                tensor=base.tensor, offset=i * P * T_act * D,
                ap=[[T_act * D, P], [1, T_act * D]],
            )
            e2 = ebuf.tile([P, T_act * D], e_dt, tag="e")
            nc.sync.dma_start(out=e2, in_=flat)
            ob = outs.tile([P, D], f32)
            nc.scalar.copy(out=ob, in_=e2[:, 0:D].bitcast(f32))
            nc.scalar.dma_start(out=out[r0 : r0 + P, :], in_=ob)
            continue

        e = ebuf.tile([P, T_act, D], e_dt, tag="e")
        if mm_dtype in ("f32", "f32rd"):
            # HWDGE, no cast (f32rd: DRAM tensor itself is declared f32r —
            # same bits as f32, so the PE single-pass multiply just reads
            # unrounded fp32; fine at our tolerance)
            if split_load:
                th = T_act // 2
                nc.sync.dma_start(
                    out=e[:, 0:th, :],
                    in_=ent[0:th, r0 : r0 + P, :].rearrange("t s d -> s t d"),
                )
                nc.scalar.dma_start(
                    out=e[:, th:T_act, :],
                    in_=ent[th:T_act, r0 : r0 + P, :].rearrange(
                        "t s d -> s t d"),
                )
            else:
                nc.sync.dma_start(
                    out=e,
                    in_=ent[:, r0 : r0 + P, :].rearrange("t s d -> s t d"),
                )
            ec = e[:].bitcast(f32) if mm_dtype == "f32rd" else e
        else:
            # SWDGE casts f32 -> e_dt
            nc.gpsimd.dma_start(
                out=e, in_=ent[:, r0 : r0 + P, :].rearrange("t s d -> s t d")
            )
            ec = e[:].bitcast(f32) if mm_dtype == "f32r" else e

        if mode == "dmaonly":
            ob = outs.tile([P, D], f32)
            nc.scalar.copy(out=ob, in_=ec[:, 0, :])
            nc.scalar.dma_start(out=out[r0 : r0 + P, :], in_=ob)
            continue

        if mode == "nostats":
            # skip stats+softmax: uniform weights straight into the diag build
            ex = stats.tile([P, T_act], f32)
            nc.vector.memset(ex, 1.0 / T_act)
            rsum = stats.tile([P, 1], f32)
            nc.vector.memset(rsum, 1.0)
            _v2_mm(nc, T_act, nh, e, ex, rsum, id_t, dg_eng, dg_dt, diagp,
                   outs, psump, out, r0, D)
            continue

        ssq = stats.tile([P, T_act], f32)
        qd = stats.tile([P, T_act], f32)
        scr_dt = e_dt if mm_dtype == "bf16" else f32
        sq_scr = scr.tile([P, D], scr_dt, tag="sq")
        qd_scr = scr.tile([P, D], scr_dt, tag="qd")
        if n_gq:
            gq_scr = scr.tile([P, D], scr_dt, tag="gq")
        else:
            gq_scr = None
        for t in range(T_act):
            nc.scalar.activation(
                out=sq_scr,
                in_=ec[:, t, :],
                func=AF.Square,
                accum_out=ssq[:, t : t + 1],
            )
            on_gp = t >= T_act - n_gq
            (nc.gpsimd if on_gp else nc.vector).scalar_tensor_tensor(
                out=(gq_scr if on_gp else qd_scr),
                in0=ec[:, t, :],
                scalar=0.0,
                in1=qb,
                op0=OP.bypass,
                op1=OP.mult,
                accum_out=qd[:, t : t + 1],
            )

        # rinv = rsqrt(ssq/D + eps) = exp(-0.5 * ln(ssq/D + eps))
        lnm = stats.tile([P, T_act], f32)
        nc.scalar.activation(
            out=lnm, in_=ssq, func=AF.Ln, scale=1.0 / D, bias=epsb[:, 0:1]
        )
        rinv = stats.tile([P, T_act], f32)
        nc.scalar.activation(out=rinv, in_=lnm, func=AF.Exp, scale=-0.5)
        lg = stats.tile([P, T_act], f32)
        nc.vector.tensor_mul(lg, qd, rinv)

        # softmax over the free (t) axis; normalization deferred to the evac
        mx = stats.tile([P, 1], f32)
        nc.vector.tensor_reduce(
            out=mx, in_=lg, axis=mybir.AxisListType.X, op=OP.max
        )
        negm = stats.tile([P, 1], f32)
        nc.vector.tensor_scalar_mul(negm, mx, -1.0)
        ex = stats.tile([P, T_act], f32)
        sume = stats.tile([P, 1], f32)
        nc.scalar.activation(
            out=ex, in_=lg, func=AF.Exp, bias=negm, accum_out=sume
        )
        rsum = stats.tile([P, 1], f32)
        nc.vector.reciprocal(rsum, sume)

        if mode == "nomm":
            # skip the weighted sum: store a plain copy (tests stats path)
            ob = outs.tile([P, D], f32)
            nc.scalar.activation(out=ob, in_=ec[:, 0, :], func=AF.Copy,
                                 scale=rsum)
            nc.scalar.dma_start(out=out[r0 : r0 + P, :], in_=ob)
            continue

        _v2_mm(nc, T_act, nh, e, ex, rsum, id_t, dg_eng, dg_dt, diagp, outs,
               psump, out, r0, D)


def _v2_mm(nc, T_act, nh, e, ex, rsum, id_t, dg_eng, dg_dt, diagp, outs,
           psump, out, r0, D):
    f32 = mybir.dt.float32
    AF = mybir.ActivationFunctionType
    OP = mybir.AluOpType
    # dg[p, t, c] = id[p, c] * ex[p, t] via stride-0 broadcast inputs
    dg_all = diagp.tile([P, T_act, P], dg_dt, tag="dg")
    ida = id_t[:, :]
    wa = ex[:, 0:T_act]
    idb = bass.AP(tensor=ida.tensor, offset=ida.offset,
                  ap=[ida.ap[0], [0, T_act], ida.ap[1]])
    wb = bass.AP(tensor=wa.tensor, offset=wa.offset,
                 ap=[wa.ap[0], wa.ap[1], [0, P]])
    dg_builder = nc.gpsimd if dg_eng == "gpsimd" else nc.vector
    dg_builder.tensor_tensor(out=dg_all, in0=idb, in1=wb, op=OP.mult)

    po = psump.tile([P, D], f32)
    for t in range(T_act):
        lhsT = dg_all[:, t, :]
        for h in range(nh):
            cs = slice(h * 512, (h + 1) * 512)
            nc.tensor.matmul(
                po[:, cs],
                lhsT=lhsT,
                rhs=e[:, t, cs],
                start=(t == 0),
                stop=(t == T_act - 1),
            )

    ob = outs.tile([P, D], f32)
    nc.scalar.activation(out=ob, in_=po, func=AF.Copy, scale=rsum)
    nc.scalar.dma_start(out=out[r0 : r0 + P, :], in_=ob)


# ---------------------------------------------------------------------------
# v4: 256-row blocks, 2 rows per partition -> 8 KB-contiguous DMA descriptors
# (vs 4 KB in v3), t-dim split into two tile-group loads to keep SBUF bounded.
# ---------------------------------------------------------------------------


def _build_v4(T_act, R, D, dg_eng="vector", ebufs=6, n_tg=4, n_gq=0,
              mode="full", split_load=False, rep=1):
    """256-row blocks (2 rows/partition -> 8 KB-contiguous DMA descriptors);
    t dim loaded as n_tg separate tile-groups so the e-buffer pool rotates
    at fine granularity (an e tile stays alive until the weighted-sum matmul
    consumes it ~30-40 us after stats start; small tiles + many bufs keep
    the load pipe from stalling on slot reuse)."""
    f32 = mybir.dt.float32
    f32r = mybir.dt.float32r
    AF = mybir.ActivationFunctionType
    OP = mybir.AluOpType
    U = 2  # rows per partition
    RB = P * U  # rows per block
    assert R % RB == 0 and D % 512 == 0 and T_act >= 2
    n_blocks = R // RB
    # t-group boundaries, e.g. T=12, n_tg=4 -> [(0,3),(3,6),(6,9),(9,12)]
    n_tg = min(n_tg, T_act)
    gsz = (T_act + n_tg - 1) // n_tg
    tg = []
    t0 = 0
    while t0 < T_act:
        tg.append((t0, min(t0 + gsz, T_act)))
        t0 += gsz
    nh = D // 512

    nc = bacc.Bacc()
    ent, qv, ident, repc, out = _declare_io(nc, T_act, R, D, False,
                                            ent_dt=f32r)

    with tile.TileContext(nc) as tc:
        with (
            tc.tile_pool(name="singles", bufs=1) as singles,
            tc.tile_pool(name="ebuf", bufs=ebufs) as ebuf,
            tc.tile_pool(name="stats", bufs=2) as stats,
            tc.tile_pool(name="scr", bufs=1) as scr,
            tc.tile_pool(name="diag", bufs=2) as diagp,
            tc.tile_pool(name="outs", bufs=2) as outs,
            tc.tile_pool(name="psum", bufs=2, space="PSUM") as psump,
        ):
            qb = singles.tile([P, D], f32)
            nc.gpsimd.dma_start(out=qb, in_=qv[:, :].to_broadcast((P, D)))
            id_t = singles.tile([P, P], f32)
            nc.sync.dma_start(out=id_t, in_=ident[:, :])
            epsb = singles.tile([P, 1], f32)
            nc.vector.memset(epsb, float(NORM_EPS))

            loop_ctx = tc.For_i(0, rep, 1) if rep > 1 else \
                contextlib.nullcontext()
            with loop_ctx:
                for i in range(n_blocks):
                    r0 = i * RB
                    egs = []
                    for gi, (g0, g1) in enumerate(tg):
                        eg = ebuf.tile([P, g1 - g0, U, D], f32r, tag="e")
                        eng = nc.scalar if (split_load and gi % 2) else \
                            nc.sync
                        eng.dma_start(
                            out=eg,
                            in_=ent[g0:g1, r0 : r0 + RB, :].rearrange(
                                "t (s u) d -> s t u d", u=U),
                        )
                        egs.append(eg)

                    def et(t, u, cs=slice(None)):
                        gi = t // gsz
                        return egs[gi][:, t - tg[gi][0], u, cs]

                    ob = outs.tile([P, U, D], f32)

                    if mode == "dmaonly":
                        nc.scalar.copy(out=ob[:, 0, :],
                                       in_=et(0, 0).bitcast(f32))
                        nc.scalar.copy(out=ob[:, 1, :],
                                       in_=et(0, 1).bitcast(f32))
                        nc.scalar.dma_start(
                            out=out[r0 : r0 + RB, :].rearrange(
                                "(s u) d -> s u d", u=U),
                            in_=ob,
                        )
                        continue

                    sq_scr = scr.tile([P, D], f32, tag="sq")
                    qd_scr = scr.tile([P, D], f32, tag="qd")
                    if n_gq:
                        gq_scr = scr.tile([P, D], f32, tag="gq")
                    for u in range(U):
                        if mode == "nostats":
                            ex = stats.tile([P, T_act], f32)
                            nc.vector.memset(ex, 1.0 / T_act)
                            rsum = stats.tile([P, 1], f32)
                            nc.vector.memset(rsum, 1.0)
                            _v4_mm(nc, T_act, nh, et, u, ex, rsum, id_t,
                                   dg_eng, diagp, psump, ob)
                            continue
                        ssq = stats.tile([P, T_act], f32)
                        qd = stats.tile([P, T_act], f32)
                        for t in range(T_act):
                            ef = et(t, u).bitcast(f32)
                            nc.scalar.activation(
                                out=sq_scr, in_=ef, func=AF.Square,
                                accum_out=ssq[:, t : t + 1],
                            )
                            on_gp = t >= T_act - n_gq
                            (nc.gpsimd if on_gp else
                             nc.vector).scalar_tensor_tensor(
                                out=(gq_scr if on_gp else qd_scr),
                                in0=ef, scalar=0.0, in1=qb,
                                op0=OP.bypass, op1=OP.mult,
                                accum_out=qd[:, t : t + 1],
                            )

                        lnm = stats.tile([P, T_act], f32)
                        nc.scalar.activation(
                            out=lnm, in_=ssq, func=AF.Ln, scale=1.0 / D,
                            bias=epsb[:, 0:1],
                        )
                        rinv = stats.tile([P, T_act], f32)
                        nc.scalar.activation(out=rinv, in_=lnm, func=AF.Exp,
                                             scale=-0.5)
                        lg = stats.tile([P, T_act], f32)
                        nc.vector.tensor_mul(lg, qd, rinv)
                        mx = stats.tile([P, 1], f32)
                        nc.vector.tensor_reduce(
                            out=mx, in_=lg, axis=mybir.AxisListType.X,
                            op=OP.max,
                        )
                        negm = stats.tile([P, 1], f32)
                        nc.vector.tensor_scalar_mul(negm, mx, -1.0)
                        ex = stats.tile([P, T_act], f32)
                        sume = stats.tile([P, 1], f32)
                        nc.scalar.activation(
                            out=ex, in_=lg, func=AF.Exp, bias=negm,
                            accum_out=sume,
                        )
                        rsum = stats.tile([P, 1], f32)
                        nc.vector.reciprocal(rsum, sume)

                        if mode == "nomm":
                            nc.scalar.activation(out=ob[:, u, :],
                                                 in_=et(0, u).bitcast(f32),
                                                 func=AF.Copy, scale=rsum)
                            continue

                        _v4_mm(nc, T_act, nh, et, u, ex, rsum, id_t, dg_eng,
                               diagp, psump, ob)

                    nc.scalar.dma_start(
                        out=out[r0 : r0 + RB, :].rearrange(
                            "(s u) d -> s u d", u=U),
                        in_=ob,
                    )

    nc.finalize()
    return nc


def _v4_mm(nc, T_act, nh, et, u, ex, rsum, id_t, dg_eng, diagp, psump, ob):
    f32 = mybir.dt.float32
    f32r = mybir.dt.float32r
    AF = mybir.ActivationFunctionType
    OP = mybir.AluOpType
    dg_all = diagp.tile([P, T_act, P], f32r, tag="dg")
    ida = id_t[:, :]
    wa = ex[:, 0:T_act]
    idb = bass.AP(tensor=ida.tensor, offset=ida.offset,
                  ap=[ida.ap[0], [0, T_act], ida.ap[1]])
    wb = bass.AP(tensor=wa.tensor, offset=wa.offset,
                 ap=[wa.ap[0], wa.ap[1], [0, P]])
    dg_builder = nc.gpsimd if dg_eng == "gpsimd" else nc.vector
    dg_builder.tensor_tensor(out=dg_all, in0=idb, in1=wb, op=OP.mult)

    po = psump.tile([P, nh * 512], f32)
    for t in range(T_act):
        lhsT = dg_all[:, t, :]
        for h in range(nh):
            cs = slice(h * 512, (h + 1) * 512)
            nc.tensor.matmul(
                po[:, cs], lhsT=lhsT, rhs=et(t, u, cs),
                start=(t == 0), stop=(t == T_act - 1),
            )

    nc.scalar.activation(out=ob[:, u, :], in_=po, func=AF.Copy, scale=rsum)


# ---------------------------------------------------------------------------
# v5 = v4 + one-block software-pipelined PSUM evacuation.  ScalarE's queue is
# strict FIFO: with the evac emitted right after a block's matmuls, ScalarE
# sits idle waiting for the softmax->diag->matmul chain before it can issue
# the evac, stalling the NEXT block's Square ops behind it.  Deferring each
# block's evac+store into the following block (PSUM: 8 single-bank
# accumulators, two blocks in flight) removes the stall.
# ---------------------------------------------------------------------------


def _build_v5(T_act, R, D, n_tg=4, ebufs=6, mode="full", rep=1):
    f32 = mybir.dt.float32
    f32r = mybir.dt.float32r
    AF = mybir.ActivationFunctionType
    OP = mybir.AluOpType
    U = 2  # rows per partition
    RB = P * U
    assert R % RB == 0 and D == 1024 and T_act >= 2
    n_blocks = R // RB
    n_tg = min(n_tg, T_act)
    gsz = (T_act + n_tg - 1) // n_tg
    tg = []
    t0 = 0
    while t0 < T_act:
        tg.append((t0, min(t0 + gsz, T_act)))
        t0 += gsz
    nh = D // 512

    nc = bacc.Bacc()
    ent, qv, ident, repc, out = _declare_io(nc, T_act, R, D, False,
                                            ent_dt=f32r)

    with tile.TileContext(nc) as tc:
        with (
            tc.tile_pool(name="singles", bufs=1) as singles,
            tc.tile_pool(name="ebuf", bufs=ebufs) as ebuf,
            tc.tile_pool(name="stats", bufs=2) as stats,
            tc.tile_pool(name="scr", bufs=1) as scr,
            tc.tile_pool(name="diag", bufs=2) as diagp,
            tc.tile_pool(name="outs", bufs=2) as outs,
            tc.tile_pool(name="psum", bufs=2, space="PSUM") as psump,
        ):
            qb = singles.tile([P, D], f32)
            nc.gpsimd.dma_start(out=qb, in_=qv[:, :].to_broadcast((P, D)))
            id_t = singles.tile([P, P], f32)
            nc.sync.dma_start(out=id_t, in_=ident[:, :])
            epsb = singles.tile([P, 1], f32)
            nc.vector.memset(epsb, float(NORM_EPS))

            def flush(pending):
                po_list, rsums, r0p = pending
                ob = outs.tile([P, U, D], f32, tag="ob")
                for u, h, po in po_list:
                    nc.scalar.activation(
                        out=ob[:, u, h * 512 : (h + 1) * 512], in_=po,
                        func=AF.Copy, scale=rsums[u],
                    )
                nc.scalar.dma_start(
                    out=out[r0p : r0p + RB, :].rearrange(
                        "(s u) d -> s u d", u=U),
                    in_=ob,
                )

            loop_ctx = tc.For_i(0, rep, 1) if rep > 1 else \
                contextlib.nullcontext()
            with loop_ctx:
                pending = None
                for i in range(n_blocks):
                    r0 = i * RB
                    egs = []
                    for gi, (g0, g1) in enumerate(tg):
                        eg = ebuf.tile([P, g1 - g0, U, D], f32r, tag="e")
                        nc.sync.dma_start(
                            out=eg,
                            in_=ent[g0:g1, r0 : r0 + RB, :].rearrange(
                                "t (s u) d -> s t u d", u=U),
                        )
                        egs.append(eg)

                    def et(t, u, cs=slice(None)):
                        gi = t // gsz
                        return egs[gi][:, t - tg[gi][0], u, cs]

                    sq_scr = scr.tile([P, D], f32, tag="sq")
                    qd_scr = scr.tile([P, D], f32, tag="qd")
                    this_po = []
                    this_rsum = {}
                    for u in range(U):
                        ssq = stats.tile([P, T_act], f32)
                        qd = stats.tile([P, T_act], f32)
                        for t in range(T_act):
                            ef = et(t, u).bitcast(f32)
                            nc.scalar.activation(
                                out=sq_scr, in_=ef, func=AF.Square,
                                accum_out=ssq[:, t : t + 1],
                            )
                            nc.vector.scalar_tensor_tensor(
                                out=qd_scr, in0=ef, scalar=0.0, in1=qb,
                                op0=OP.bypass, op1=OP.mult,
                                accum_out=qd[:, t : t + 1],
                            )

                        lnm = stats.tile([P, T_act], f32)
                        nc.scalar.activation(
                            out=lnm, in_=ssq, func=AF.Ln, scale=1.0 / D,
                            bias=epsb[:, 0:1],
                        )
                        rinv = stats.tile([P, T_act], f32)
                        nc.scalar.activation(out=rinv, in_=lnm, func=AF.Exp,
                                             scale=-0.5)
                        lg = stats.tile([P, T_act], f32)
                        nc.vector.tensor_mul(lg, qd, rinv)
                        mx = stats.tile([P, 1], f32)
                        nc.vector.tensor_reduce(
                            out=mx, in_=lg, axis=mybir.AxisListType.X,
                            op=OP.max,
                        )
                        negm = stats.tile([P, 1], f32)
                        nc.vector.tensor_scalar_mul(negm, mx, -1.0)
                        ex = stats.tile([P, T_act], f32)
                        sume = stats.tile([P, 1], f32)
                        nc.scalar.activation(
                            out=ex, in_=lg, func=AF.Exp, bias=negm,
                            accum_out=sume,
                        )
                        rsum = stats.tile([P, 1], f32, tag=f"rs{u}")
                        nc.vector.reciprocal(rsum, sume)
                        this_rsum[u] = rsum

                        if mode == "nodep":
                            # same work, severed dependency: dg consumes a
                            # memset instead of the softmax output
                            ex = stats.tile([P, T_act], f32, tag=f"exm{u}")
                            nc.vector.memset(ex, 1.0 / T_act)

                        dg_all = diagp.tile([P, T_act, P], f32r, tag="dg")
                        ida = id_t[:, :]
                        wa = ex[:, 0:T_act]
                        idb = bass.AP(tensor=ida.tensor, offset=ida.offset,
                                      ap=[ida.ap[0], [0, T_act], ida.ap[1]])
                        wb = bass.AP(tensor=wa.tensor, offset=wa.offset,
                                     ap=[wa.ap[0], wa.ap[1], [0, P]])
                        nc.vector.tensor_tensor(out=dg_all, in0=idb, in1=wb,
                                                op=OP.mult)

                        pos = []
                        for h in range(nh):
                            po = psump.tile([P, 512], f32, tag=f"po{u}{h}")
                            pos.append(po)
                        for t in range(T_act):
                            lhsT = dg_all[:, t, :]
                            for h in range(nh):
                                cs = slice(h * 512, (h + 1) * 512)
                                nc.tensor.matmul(
                                    pos[h], lhsT=lhsT, rhs=et(t, u, cs),
                                    start=(t == 0), stop=(t == T_act - 1),
                                )
                        for h in range(nh):
                            this_po.append((u, h, pos[h]))

                    if pending is not None:
                        flush(pending)
                    pending = (this_po, this_rsum, r0)
                flush(pending)

    nc.finalize()
    return nc


# ---------------------------------------------------------------------------
# v6 = v5 + stats interleaved across the two row-chains (u) per arriving
# tile-group, finer 2-t tile-groups with a deep e-buffer pool.  The e-tiles
# of a block stay alive until the block's LAST matmul; the post-load tail
# (remaining qdots -> softmax -> diag -> matmul) is what sets tile lifetime,
# so interleaving u keeps the tail short and 10 small buffers cover it.
# ---------------------------------------------------------------------------


def _build_v6(T_act, R, D, n_tg=6, ebufs=10, rep=1):
    f32 = mybir.dt.float32
    f32r = mybir.dt.float32r
    bf16 = mybir.dt.bfloat16
    AF = mybir.ActivationFunctionType
    OP = mybir.AluOpType
    U = 2  # rows per partition
    RB = P * U
    assert R % RB == 0 and D == 1024 and T_act >= 2
    n_blocks = R // RB
    n_tg = min(n_tg, T_act)
    gsz = (T_act + n_tg - 1) // n_tg
    tg = []
    t0 = 0
    while t0 < T_act:
        tg.append((t0, min(t0 + gsz, T_act)))
        t0 += gsz
    nh = D // 512

    nc = bacc.Bacc()
    ent, qv, ident, repc, out = _declare_io(nc, T_act, R, D, False,
                                            ent_dt=f32r)

    with tile.TileContext(nc) as tc:
        with (
            tc.tile_pool(name="singles", bufs=1) as singles,
            tc.tile_pool(name="ebuf", bufs=ebufs) as ebuf,
            tc.tile_pool(name="stats", bufs=2) as stats,
            tc.tile_pool(name="scr", bufs=1) as scr,
            tc.tile_pool(name="diag", bufs=2) as diagp,
            tc.tile_pool(name="outs", bufs=2) as outs,
            tc.tile_pool(name="psum", bufs=2, space="PSUM") as psump,
        ):
            qb = singles.tile([P, D], f32)
            nc.gpsimd.dma_start(out=qb, in_=qv[:, :].to_broadcast((P, D)))
            id_t = singles.tile([P, P], f32)
            nc.sync.dma_start(out=id_t, in_=ident[:, :])
            epsb = singles.tile([P, 1], f32)
            nc.vector.memset(epsb, float(NORM_EPS))

            def flush(pending):
                po_list, rsums, r0p = pending
                ob = outs.tile([P, U, D], f32, tag="ob")
                for u, h, po in po_list:
                    nc.scalar.activation(
                        out=ob[:, u, h * 512 : (h + 1) * 512], in_=po,
                        func=AF.Copy, scale=rsums[u],
                    )
                nc.scalar.dma_start(
                    out=out[r0p : r0p + RB, :].rearrange(
                        "(s u) d -> s u d", u=U),
                    in_=ob,
                )

            loop_ctx = tc.For_i(0, rep, 1) if rep > 1 else \
                contextlib.nullcontext()
            with loop_ctx:
                pending = None
                for i in range(n_blocks):
                    r0 = i * RB
                    egs = []
                    for gi, (g0, g1) in enumerate(tg):
                        eg = ebuf.tile([P, g1 - g0, U, D], f32r, tag="e")
                        nc.sync.dma_start(
                            out=eg,
                            in_=ent[g0:g1, r0 : r0 + RB, :].rearrange(
                                "t (s u) d -> s t u d", u=U),
                        )
                        egs.append(eg)

                    def et(t, u, cs=slice(None)):
                        gi = t // gsz
                        return egs[gi][:, t - tg[gi][0], u, cs]

                    sq_scr = scr.tile([P, D], bf16, tag="sq")
                    qd_scr = scr.tile([P, D], bf16, tag="qd")
                    ssqs = [stats.tile([P, T_act], f32, tag=f"ssq{u}",
                                       name=f"ssq{u}") for u in range(U)]
                    qds = [stats.tile([P, T_act], f32, tag=f"qd{u}",
                                      name=f"qd{u}") for u in range(U)]
                    # stats interleaved in tile-arrival order
                    for t in range(T_act):
                        for u in range(U):
                            ef = et(t, u).bitcast(f32)
                            nc.scalar.activation(
                                out=sq_scr, in_=ef, func=AF.Square,
                                accum_out=ssqs[u][:, t : t + 1],
                            )
                            nc.vector.scalar_tensor_tensor(
                                out=qd_scr, in0=ef, scalar=0.0, in1=qb,
                                op0=OP.bypass, op1=OP.mult,
                                accum_out=qds[u][:, t : t + 1],
                            )

                    this_po = []
                    this_rsum = {}
                    for u in range(U):
                        lnm = stats.tile([P, T_act], f32)
                        nc.scalar.activation(
                            out=lnm, in_=ssqs[u], func=AF.Ln, scale=1.0 / D,
                            bias=epsb[:, 0:1],
                        )
                        rinv = stats.tile([P, T_act], f32)
                        nc.scalar.activation(out=rinv, in_=lnm, func=AF.Exp,
                                             scale=-0.5)
                        lg = stats.tile([P, T_act], f32)
                        nc.vector.tensor_mul(lg, qds[u], rinv)
                        mx = stats.tile([P, 1], f32)
                        nc.vector.tensor_reduce(
                            out=mx, in_=lg, axis=mybir.AxisListType.X,
                            op=OP.max,
                        )
                        negm = stats.tile([P, 1], f32)
                        nc.vector.tensor_scalar_mul(negm, mx, -1.0)
                        ex = stats.tile([P, T_act], f32)
                        sume = stats.tile([P, 1], f32)
                        nc.scalar.activation(
                            out=ex, in_=lg, func=AF.Exp, bias=negm,
                            accum_out=sume,
                        )
                        rsum = stats.tile([P, 1], f32, tag=f"rs{u}")
                        nc.vector.reciprocal(rsum, sume)
                        this_rsum[u] = rsum

                        dg_all = diagp.tile([P, T_act, P], f32r, tag="dg")
                        ida = id_t[:, :]
                        wa = ex[:, 0:T_act]
                        idb = bass.AP(tensor=ida.tensor, offset=ida.offset,
                                      ap=[ida.ap[0], [0, T_act], ida.ap[1]])
                        wb = bass.AP(tensor=wa.tensor, offset=wa.offset,
                                     ap=[wa.ap[0], wa.ap[1], [0, P]])
                        nc.vector.tensor_tensor(out=dg_all, in0=idb, in1=wb,
                                                op=OP.mult)

                        pos = []
                        for h in range(nh):
                            po = psump.tile([P, 512], f32, tag=f"po{u}{h}")
                            pos.append(po)
                        for t in range(T_act):
                            lhsT = dg_all[:, t, :]
                            for h in range(nh):
                                cs = slice(h * 512, (h + 1) * 512)
                                nc.tensor.matmul(
                                    pos[h], lhsT=lhsT, rhs=et(t, u, cs),
                                    start=(t == 0), stop=(t == T_act - 1),
                                )
                        for h in range(nh):
                            this_po.append((u, h, pos[h]))

                    if pending is not None:
                        flush(pending)
                    pending = (this_po, this_rsum, r0)
                flush(pending)

    nc.finalize()
    return nc


# ---------------------------------------------------------------------------
# v7 = v6 with the per-u softmax smalls merged into single wide ops over a
# u-major [P, U*T] stats layout (one Ln, one Exp, one mul, one strided max-
# reduce, one broadcast subtract, one strided sum-reduce, one reciprocal per
# block instead of 2x each) — fewer small-op overheads and fewer ScalarE<->
# VectorE ping-pongs in the post-load tail.
# ---------------------------------------------------------------------------


def _build_v7(T_act, R, D, n_tg=6, ebufs=10, dg_eng="vector", pin_tables=True,
              obufs=2, dbufs=2, rep=1):
    f32 = mybir.dt.float32
    f32r = mybir.dt.float32r
    bf16 = mybir.dt.bfloat16
    AF = mybir.ActivationFunctionType
    OP = mybir.AluOpType
    U = 2
    RB = P * U
    assert R % RB == 0 and D == 1024 and T_act >= 2
    n_blocks = R // RB
    n_tg = min(n_tg, T_act)
    gsz = (T_act + n_tg - 1) // n_tg
    tg = []
    t0 = 0
    while t0 < T_act:
        tg.append((t0, min(t0 + gsz, T_act)))
        t0 += gsz
    nh = D // 512

    nc = bacc.Bacc()
    if pin_tables:
        _pin_act_tables(nc)
    ent, qv, ident, repc, out = _declare_io(nc, T_act, R, D, False,
                                            ent_dt=f32r)

    with tile.TileContext(nc) as tc:
        with (
            tc.tile_pool(name="singles", bufs=1) as singles,
            tc.tile_pool(name="ebuf", bufs=ebufs) as ebuf,
            tc.tile_pool(name="stats", bufs=2) as stats,
            tc.tile_pool(name="scr", bufs=1) as scr,
            tc.tile_pool(name="diag", bufs=dbufs) as diagp,
            tc.tile_pool(name="outs", bufs=obufs) as outs,
            tc.tile_pool(name="psum", bufs=2, space="PSUM") as psump,
        ):
            qb = singles.tile([P, D], f32)
            nc.gpsimd.dma_start(out=qb, in_=qv[:, :].to_broadcast((P, D)))
            id_t = singles.tile([P, P], f32)
            nc.sync.dma_start(out=id_t, in_=ident[:, :])
            epsb = singles.tile([P, 1], f32)
            nc.vector.memset(epsb, float(NORM_EPS))

            def flush(pending):
                po_list, rsum_p, r0p = pending
                ob = outs.tile([P, U, D], f32, tag="ob")
                for u, h, po in po_list:
                    nc.scalar.activation(
                        out=ob[:, u, h * 512 : (h + 1) * 512], in_=po,
                        func=AF.Copy, scale=rsum_p[:, u : u + 1],
                    )
                nc.scalar.dma_start(
                    out=out[r0p : r0p + RB, :].rearrange(
                        "(s u) d -> s u d", u=U),
                    in_=ob,
                )

            loop_ctx = tc.For_i(0, rep, 1) if rep > 1 else \
                contextlib.nullcontext()
            with loop_ctx:
                pending = None
                for i in range(n_blocks):
                    r0 = i * RB
                    egs = []
                    for gi, (g0, g1) in enumerate(tg):
                        eg = ebuf.tile([P, g1 - g0, U, D], f32r, tag="e")
                        nc.sync.dma_start(
                            out=eg,
                            in_=ent[g0:g1, r0 : r0 + RB, :].rearrange(
                                "t (s u) d -> s t u d", u=U),
                        )
                        egs.append(eg)

                    def et(t, u, cs=slice(None)):
                        gi = t // gsz
                        return egs[gi][:, t - tg[gi][0], u, cs]

                    sq_scr = scr.tile([P, D], bf16, tag="sq")
                    qd_scr = scr.tile([P, D], bf16, tag="qd")
                    # u-major stats: column u*T_act + t
                    ssq = stats.tile([P, U * T_act], f32, tag="ssq")
                    qd = stats.tile([P, U * T_act], f32, tag="qd")
                    for t in range(T_act):
                        for u in range(U):
                            c = u * T_act + t
                            ef = et(t, u).bitcast(f32)
                            nc.scalar.activation(
                                out=sq_scr, in_=ef, func=AF.Square,
                                accum_out=ssq[:, c : c + 1],
                            )
                            nc.vector.scalar_tensor_tensor(
                                out=qd_scr, in0=ef, scalar=0.0, in1=qb,
                                op0=OP.bypass, op1=OP.mult,
                                accum_out=qd[:, c : c + 1],
                            )

                    # merged softmax smalls over [P, U*T]
                    lnm = stats.tile([P, U * T_act], f32)
                    nc.scalar.activation(
                        out=lnm, in_=ssq, func=AF.Ln, scale=1.0 / D,
                        bias=epsb[:, 0:1],
                    )
                    rinv = stats.tile([P, U * T_act], f32)
                    nc.scalar.activation(out=rinv, in_=lnm, func=AF.Exp,
                                         scale=-0.5)
                    lg = stats.tile([P, U * T_act], f32)
                    nc.vector.tensor_mul(lg, qd, rinv)
                    lg3 = lg[:].rearrange("p (u t) -> p u t", u=U)
                    mx = stats.tile([P, U], f32)
                    nc.vector.tensor_reduce(
                        out=mx, in_=lg3, axis=mybir.AxisListType.X, op=OP.max
                    )
                    # lgc = lg - mx  (mx broadcast over t via stride-0 AP)
                    mxa = mx[:, :]
                    mxb = bass.AP(tensor=mxa.tensor, offset=mxa.offset,
                                  ap=[mxa.ap[0], mxa.ap[1], [0, T_act]])
                    lgc = stats.tile([P, U * T_act], f32, tag="lgc")
                    nc.vector.tensor_tensor(
                        out=lgc[:].rearrange("p (u t) -> p u t", u=U),
                        in0=lg3, in1=mxb, op=OP.subtract,
                    )
                    ex = stats.tile([P, U * T_act], f32, tag="ex")
                    nc.scalar.activation(out=ex, in_=lgc, func=AF.Exp)
                    sume = stats.tile([P, U], f32)
                    nc.vector.tensor_reduce(
                        out=sume,
                        in_=ex[:].rearrange("p (u t) -> p u t", u=U),
                        axis=mybir.AxisListType.X, op=OP.add,
                    )
                    rsum = stats.tile([P, U], f32, tag="rsum")
                    nc.vector.reciprocal(rsum, sume)

                    this_po = []
                    for u in range(U):
                        dg_all = diagp.tile([P, T_act, P], f32r, tag="dg")
                        ida = id_t[:, :]
                        wa = ex[:, u * T_act : (u + 1) * T_act]
                        idb = bass.AP(tensor=ida.tensor, offset=ida.offset,
                                      ap=[ida.ap[0], [0, T_act], ida.ap[1]])
                        wb = bass.AP(tensor=wa.tensor, offset=wa.offset,
                                     ap=[wa.ap[0], wa.ap[1], [0, P]])
                        dg_builder = (nc.gpsimd if dg_eng == "gpsimd"
                                      else nc.vector)
                        dg_builder.tensor_tensor(out=dg_all, in0=idb, in1=wb,
                                                 op=OP.mult)

                        pos = []
                        for h in range(nh):
                            po = psump.tile([P, 512], f32, tag=f"po{u}{h}")
                            pos.append(po)
                        for t in range(T_act):
                            lhsT = dg_all[:, t, :]
                            for h in range(nh):
                                cs = slice(h * 512, (h + 1) * 512)
                                nc.tensor.matmul(
                                    pos[h], lhsT=lhsT, rhs=et(t, u, cs),
                                    start=(t == 0), stop=(t == T_act - 1),
                                )
                        for h in range(nh):
                            this_po.append((u, h, pos[h]))

                    if pending is not None:
                        flush(pending)
                    pending = (this_po, rsum, r0)
                flush(pending)

    nc.finalize()
    return nc


# ---------------------------------------------------------------------------
# v8 = v7 with the Ln-based rsqrt replaced by the integer-seed Newton rsqrt
# on VectorE.  Using Ln makes the act-table-set assigner flip-flop between
# two table sets (Square/Exp -> exp_and_others, Ln -> natural_log_...),
# inserting a ~2.7 us InstLoadActFuncSet TWICE PER BLOCK (~42 us/iter, and
# it sits in the softmax dependency tail).  With only Square/Exp/Copy the
# whole kernel needs ONE table load.
# ---------------------------------------------------------------------------


def _build_v8(T_act, R, D, n_tg=6, ebufs=10, newton=2, rep=1):
    f32 = mybir.dt.float32
    f32r = mybir.dt.float32r
    u32 = mybir.dt.uint32
    bf16 = mybir.dt.bfloat16
    AF = mybir.ActivationFunctionType
    OP = mybir.AluOpType
    U = 2
    RB = P * U
    assert R % RB == 0 and D == 1024 and T_act >= 2
    n_blocks = R // RB
    n_tg = min(n_tg, T_act)
    gsz = (T_act + n_tg - 1) // n_tg
    tg = []
    t0 = 0
    while t0 < T_act:
        tg.append((t0, min(t0 + gsz, T_act)))
        t0 += gsz
    nh = D // 512
    UT = U * T_act

    nc = bacc.Bacc()
    ent, qv, ident, repc, out = _declare_io(nc, T_act, R, D, False,
                                            ent_dt=f32r)

    with tile.TileContext(nc) as tc:
        with (
            tc.tile_pool(name="singles", bufs=1) as singles,
            tc.tile_pool(name="ebuf", bufs=ebufs) as ebuf,
            tc.tile_pool(name="stats", bufs=2) as stats,
            tc.tile_pool(name="scr", bufs=1) as scr,
            tc.tile_pool(name="diag", bufs=2) as diagp,
            tc.tile_pool(name="outs", bufs=2) as outs,
            tc.tile_pool(name="psum", bufs=2, space="PSUM") as psump,
        ):
            qb = singles.tile([P, D], f32)
            nc.gpsimd.dma_start(out=qb, in_=qv[:, :].to_broadcast((P, D)))
            id_t = singles.tile([P, P], f32)
            nc.sync.dma_start(out=id_t, in_=ident[:, :])

            def flush(pending):
                po_list, rsum_p, r0p = pending
                ob = outs.tile([P, U, D], f32, tag="ob")
                for u, h, po in po_list:
                    nc.scalar.activation(
                        out=ob[:, u, h * 512 : (h + 1) * 512], in_=po,
                        func=AF.Copy, scale=rsum_p[:, u : u + 1],
                    )
                nc.scalar.dma_start(
                    out=out[r0p : r0p + RB, :].rearrange(
                        "(s u) d -> s u d", u=U),
                    in_=ob,
                )

            loop_ctx = tc.For_i(0, rep, 1) if rep > 1 else \
                contextlib.nullcontext()
            with loop_ctx:
                pending = None
                for i in range(n_blocks):
                    r0 = i * RB
                    egs = []
                    for gi, (g0, g1) in enumerate(tg):
                        eg = ebuf.tile([P, g1 - g0, U, D], f32r, tag="e")
                        nc.sync.dma_start(
                            out=eg,
                            in_=ent[g0:g1, r0 : r0 + RB, :].rearrange(
                                "t (s u) d -> s t u d", u=U),
                        )
                        egs.append(eg)

                    def et(t, u, cs=slice(None)):
                        gi = t // gsz
                        return egs[gi][:, t - tg[gi][0], u, cs]

                    sq_scr = scr.tile([P, D], bf16, tag="sq")
                    qd_scr = scr.tile([P, D], bf16, tag="qd")
                    # u-major stats: column u*T_act + t
                    ssq = stats.tile([P, UT], f32, tag="ssq")
                    qd = stats.tile([P, UT], f32, tag="qd")
                    for t in range(T_act):
                        for u in range(U):
                            c = u * T_act + t
                            ef = et(t, u).bitcast(f32)
                            nc.scalar.activation(
                                out=sq_scr, in_=ef, func=AF.Square,
                                accum_out=ssq[:, c : c + 1],
                            )
                            nc.vector.scalar_tensor_tensor(
                                out=qd_scr, in0=ef, scalar=0.0, in1=qb,
                                op0=OP.bypass, op1=OP.mult,
                                accum_out=qd[:, c : c + 1],
                            )

                    # rinv = rsqrt(ssq/D + eps): integer-seed Newton on DVE
                    ms = stats.tile([P, UT], f32)
                    nc.vector.tensor_scalar(
                        out=ms, in0=ssq, scalar1=1.0 / D,
                        scalar2=float(NORM_EPS), op0=OP.mult, op1=OP.add,
                    )
                    sh = stats.tile([P, UT], u32)
                    nc.vector.tensor_scalar(
                        out=sh, in0=ms[:].bitcast(u32), scalar1=1,
                        scalar2=None, op0=OP.logical_shift_right,
                    )
                    shf = stats.tile([P, UT], f32)
                    nc.vector.tensor_copy(shf, sh)
                    nc.vector.tensor_scalar(
                        out=shf, in0=shf, scalar1=-1.0,
                        scalar2=float(0x5F3759DF), op0=OP.mult, op1=OP.add,
                    )
                    yb = stats.tile([P, UT], u32)
                    nc.vector.tensor_copy(yb, shf)
                    rinv = yb[:].bitcast(f32)
                    nwt = stats.tile([P, UT], f32)
                    for _ in range(newton):
                        nc.vector.tensor_mul(nwt, rinv, rinv)
                        nc.vector.tensor_mul(nwt, nwt, ms)
                        nc.vector.tensor_scalar(
                            out=nwt, in0=nwt, scalar1=-0.5, scalar2=1.5,
                            op0=OP.mult, op1=OP.add,
                        )
                        nc.vector.tensor_mul(rinv, rinv, nwt)

                    lg = stats.tile([P, UT], f32)
                    nc.vector.tensor_mul(lg, qd, rinv)
                    lg3 = lg[:].rearrange("p (u t) -> p u t", u=U)
                    mx = stats.tile([P, U], f32)
                    nc.vector.tensor_reduce(
                        out=mx, in_=lg3, axis=mybir.AxisListType.X, op=OP.max
                    )
                    mxa = mx[:, :]
                    mxb = bass.AP(tensor=mxa.tensor, offset=mxa.offset,
                                  ap=[mxa.ap[0], mxa.ap[1], [0, T_act]])
                    lgc = stats.tile([P, UT], f32, tag="lgc")
                    nc.vector.tensor_tensor(
                        out=lgc[:].rearrange("p (u t) -> p u t", u=U),
                        in0=lg3, in1=mxb, op=OP.subtract,
                    )
                    ex = stats.tile([P, UT], f32, tag="ex")
                    nc.scalar.activation(out=ex, in_=lgc, func=AF.Exp)
                    sume = stats.tile([P, U], f32)
                    nc.vector.tensor_reduce(
                        out=sume,
                        in_=ex[:].rearrange("p (u t) -> p u t", u=U),
                        axis=mybir.AxisListType.X, op=OP.add,
                    )
                    rsum = stats.tile([P, U], f32, tag="rsum")
                    nc.vector.reciprocal(rsum, sume)

                    this_po = []
                    for u in range(U):
                        dg_all = diagp.tile([P, T_act, P], f32r, tag="dg")
                        ida = id_t[:, :]
                        wa = ex[:, u * T_act : (u + 1) * T_act]
                        idb = bass.AP(tensor=ida.tensor, offset=ida.offset,
                                      ap=[ida.ap[0], [0, T_act], ida.ap[1]])
                        wb = bass.AP(tensor=wa.tensor, offset=wa.offset,
                                     ap=[wa.ap[0], wa.ap[1], [0, P]])
                        nc.vector.tensor_tensor(out=dg_all, in0=idb, in1=wb,
                                                op=OP.mult)

                        pos = []
                        for h in range(nh):
                            po = psump.tile([P, 512], f32, tag=f"po{u}{h}")
                            pos.append(po)
                        for t in range(T_act):
                            lhsT = dg_all[:, t, :]
                            for h in range(nh):
                                cs = slice(h * 512, (h + 1) * 512)
                                nc.tensor.matmul(
                                    pos[h], lhsT=lhsT, rhs=et(t, u, cs),
                                    start=(t == 0), stop=(t == T_act - 1),
                                )
                        for h in range(nh):
                            this_po.append((u, h, pos[h]))

                    if pending is not None:
                        flush(pending)
                    pending = (this_po, rsum, r0)
                flush(pending)

    nc.finalize()
    return nc


# ---------------------------------------------------------------------------
# v1 builder (previous session's kernel), kept for A/B benchmarking.
# ---------------------------------------------------------------------------


def _build_kernel(T_act, R, D, pe_dtype="f32", n_dve=0, n_gps=0, rep=1,
                  mode="full", bench_rep=False):
    f32 = mybir.dt.float32
    assert R % P == 0 and D % 512 == 0
    n_tiles = R // P
    nh = D // 512

    nc = bacc.Bacc()
    ent, qv, ident, repc, out = _declare_io(nc, T_act, R, D, bench_rep)

    with tile.TileContext(nc) as tc:
        with (
            tc.tile_pool(name="singles", bufs=1) as singles,
            tc.tile_pool(name="ebuf", bufs=2) as ebuf,
            tc.tile_pool(name="stats", bufs=2) as stats,
            tc.tile_pool(name="scr", bufs=2) as scr,
            tc.tile_pool(name="diag", bufs=3) as diagp,
            tc.tile_pool(name="outs", bufs=3) as outs,
            tc.tile_pool(name="psum", bufs=2, space="PSUM") as psump,
        ):
            qb = singles.tile([P, D], f32)
            nc.gpsimd.dma_start(out=qb, in_=qv[:, :].to_broadcast((P, D)))
            id_t = singles.tile([P, P], f32)
            nc.sync.dma_start(out=id_t, in_=ident[:, :])

            if bench_rep:
                rt = singles.tile([1, 1], mybir.dt.uint32)
                nc.sync.dma_start(out=rt, in_=repc[:, :])
                _, (repv,) = nc.values_load_multi_w_load_instructions(
                    rt[0:1, 0:1], min_val=1, max_val=1 << 20
                )
                loop_ctx = tc.For_i(0, repv, 1)
            elif rep > 1:
                loop_ctx = tc.For_i(0, rep, 1)
            else:
                loop_ctx = contextlib.nullcontext()
            with loop_ctx:
                _loop_body(
                    nc, tc, T_act, D, n_tiles, nh, pe_dtype, n_dve, n_gps,
                    ent, out, qb, id_t, ebuf, stats, scr, diagp, outs, psump,
                    mode,
                )

    nc.finalize()
    return nc


def _loop_body(
    nc, tc, T_act, D, n_tiles, nh, pe_dtype, n_dve, n_gps,
    ent, out, qb, id_t, ebuf, stats, scr, diagp, outs, psump,
    mode="full",
):
    f32 = mybir.dt.float32
    AF = mybir.ActivationFunctionType
    OP = mybir.AluOpType
    for i in range(n_tiles):
        r0 = i * P
        e = ebuf.tile([P, T_act, D], f32, tag="e")
        nc.sync.dma_start(
            out=e,
            in_=ent[:, r0 : r0 + P, :].rearrange("t s d -> s t d"),
        )

        if mode == "dmaonly":
            ob = outs.tile([P, D], f32)
            nc.scalar.copy(out=ob, in_=e[:, 0, :])
            nc.scalar.dma_start(out=out[r0 : r0 + P, :], in_=ob)
            continue

        ssq = stats.tile([P, T_act], f32)
        qd = stats.tile([P, T_act], f32)
        sq_scr = scr.tile([P, D], f32)
        qd_scr = scr.tile([P, D], f32)
        for t in range(T_act):
            nc.scalar.activation(
                out=sq_scr,
                in_=e[:, t, :],
                func=AF.Square,
                accum_out=ssq[:, t : t + 1],
            )
            nc.vector.scalar_tensor_tensor(
                out=qd_scr,
                in0=e[:, t, :],
                scalar=0.0,
                in1=qb,
                op0=OP.bypass,
                op1=OP.mult,
                accum_out=qd[:, t : t + 1],
            )

        # ms = ssq/D + eps; rinv = rsqrt(ms) via integer-seed Newton
        ms = stats.tile([P, T_act], f32)
        nc.vector.tensor_scalar(
            out=ms,
            in0=ssq,
            scalar1=1.0 / D,
            scalar2=float(NORM_EPS),
            op0=OP.mult,
            op1=OP.add,
        )
        u32 = mybir.dt.uint32
        sh = stats.tile([P, T_act], u32)
        nc.vector.tensor_scalar(
            out=sh,
            in0=ms[:].bitcast(u32),
            scalar1=1,
            scalar2=None,
            op0=OP.logical_shift_right,
        )
        shf = stats.tile([P, T_act], f32)
        nc.vector.tensor_copy(shf, sh)
        nc.vector.tensor_scalar(
            out=shf,
            in0=shf,
            scalar1=-1.0,
            scalar2=float(0x5F3759DF),
            op0=OP.mult,
            op1=OP.add,
        )
        yb = stats.tile([P, T_act], u32)
        nc.vector.tensor_copy(yb, shf)
        rinv = yb[:].bitcast(f32)
        nwt = stats.tile([P, T_act], f32)
        for _ in range(2):
            nc.vector.tensor_mul(nwt, rinv, rinv)
            nc.vector.tensor_mul(nwt, nwt, ms)
            nc.vector.tensor_scalar(
                out=nwt,
                in0=nwt,
                scalar1=-0.5,
                scalar2=1.5,
                op0=OP.mult,
                op1=OP.add,
            )
            nc.vector.tensor_mul(rinv, rinv, nwt)

        lg = stats.tile([P, T_act], f32)
        nc.vector.tensor_mul(lg, qd, rinv)

        mx = stats.tile([P, 1], f32)
        nc.vector.tensor_reduce(
            out=mx, in_=lg, axis=mybir.AxisListType.X, op=OP.max
        )
        negm = stats.tile([P, 1], f32)
        nc.vector.tensor_scalar_mul(negm, mx, -1.0)
        ex = stats.tile([P, T_act], f32)
        sume = stats.tile([P, 1], f32)
        nc.scalar.activation(
            out=ex, in_=lg, func=AF.Exp, bias=negm, accum_out=sume
        )
        rsum = stats.tile([P, 1], f32)
        nc.vector.reciprocal(rsum, sume)
        w = stats.tile([P, T_act], f32)
        nc.vector.tensor_scalar_mul(w, ex, rsum)

        _p3(nc, T_act, D, nh, pe_dtype, n_dve, n_gps, e, w, id_t,
            diagp, outs, psump, out, r0)


def _p3(nc, T_act, D, nh, pe_dtype, n_dve, n_gps, e, w, id_t, diagp, outs,
        psump, out, r0):
    f32 = mybir.dt.float32
    OP = mybir.AluOpType
    n_pe = T_act - n_dve - n_gps
    assert n_pe >= 1
    f32r = mybir.dt.float32r
    dg_all = diagp.tile([P, n_pe, P], f32, tag="dg")
    ida = id_t[:, :]
    wa = w[:, 0:n_pe]
    idb = bass.AP(tensor=ida.tensor, offset=ida.offset,
                  ap=[ida.ap[0], [0, n_pe], ida.ap[1]])
    wb = bass.AP(tensor=wa.tensor, offset=wa.offset,
                 ap=[wa.ap[0], wa.ap[1], [0, P]])
    nc.vector.tensor_tensor(out=dg_all, in0=idb, in1=wb, op=OP.mult)

    po = psump.tile([P, D], f32)
    for h in range(nh):
        cs = slice(h * 512, (h + 1) * 512)
        for t in range(n_pe):
            lhsT = dg_all[:, t, :]
            rhs = e[:, t, cs]
            if pe_dtype == "f32r":
                lhsT = lhsT.bitcast(f32r)
                rhs = rhs.bitcast(f32r)
            nc.tensor.matmul(
                po[:, cs],
                lhsT=lhsT,
                rhs=rhs,
                start=(t == 0),
                stop=(t == n_pe - 1),
            )

    ob = outs.tile([P, D], f32)
    nc.scalar.copy(out=ob, in_=po)
    for j, t in enumerate(range(n_pe, T_act)):
        eng = nc.vector if j < n_dve else nc.gpsimd
        eng.scalar_tensor_tensor(
            out=ob,
            in0=e[:, t, :],
            scalar=w[:, t : t + 1],
            in1=ob,
            op0=OP.mult,
            op1=OP.add,
        )
    nc.scalar.dma_start(out=out[r0 : r0 + P, :], in_=ob)


def _get_kernel(T_act, R, D):
    key = (T_act, R, D)
    if key not in _kernel_cache:
        if T_act == 12 and R % 256 == 0 and D == 1024:
            # tuned fast path (the graded shape): ~347 us/core on 8x trn2,
            # DMA floor for this layout is ~321 us
            _kernel_cache[key] = _build_v7(T_act, R, D, n_tg=12, ebufs=18)
        else:
            # correctness fallback for other shapes (e.g. n_active <= 0)
            _kernel_cache[key] = _build_v2(T_act, R, D, mm_dtype="f32rd",
                                           dg_eng="vector", ebufs=2)
    return _kernel_cache[key]


def kernel(entries, proj, norm_scale, n_active, block_idx):
    entries = np.asarray(entries)
    proj = np.asarray(proj, dtype=np.float32)
    norm_scale = np.asarray(norm_scale, dtype=np.float32)
    if entries.dtype != np.float32:
        entries = entries.astype(np.float32)
    maxT, B, S, D = entries.shape
    na = int(np.asarray(n_active))
    bi = int(np.asarray(block_idx))

    if na <= 0:
        # everything masked -> softmax of equal (-1e9) logits = uniform mean
        T_act = maxT
        qprime = np.zeros((D,), dtype=np.float32)
    else:
        T_act = min(na, maxT)
        qprime = (proj[min(bi, maxT - 1)] * norm_scale).astype(np.float32)

    rows = B * S
    assert rows % (N_CORES * P) == 0, f"rows={rows} not divisible by {N_CORES * P}"
    R = rows // N_CORES

    ent_flat = entries[:T_act].reshape(T_act, rows, D)
    ident = np.eye(P, dtype=np.float32)
    qv = qprime.reshape(1, D)

    nc = _get_kernel(T_act, R, D)

    in_maps = []
    for c in range(N_CORES):
        in_maps.append(
            {
                "ent": np.ascontiguousarray(ent_flat[:, c * R : (c + 1) * R, :]),
                "qv": qv,
                "ident": ident,
            }
        )

    res = run_bass_kernel_spmd(nc, in_maps, list(range(N_CORES)))
    global _last_results
    _last_results = res
    parts = [res.results[c]["out"] for c in range(N_CORES)]
    return np.concatenate(parts, axis=0).reshape(B, S, D)


_last_results = None


# revision 47
# speedup vs baseline: 1.0451x; 1.0451x over previous
"""Depth-wise attention over block outputs (AttentionResidual) on 8 trn2 cores.

Computation (reference):
    q' = proj[min(block_idx, maxT-1)] * norm_scale
    rms[t,r]   = sqrt(mean_d(e[t,r,:]^2) + 1e-5)
    logit[t,r] = (q' . e[t,r,:]) / rms[t,r]          (masked to -1e9 for t >= n_active)
    w = softmax_t(logit);  out[r,:] = sum_t w[t,r] * e[t,r,:]

Masked entries (t >= n_active) get softmax weight exp(-1e9 - max) == 0.0 exactly
in fp32, so only the first n_active depth slices are ever read.

Sharding: flattened B*S rows are split evenly across the 8 cores (data
parallel); q' and the identity matrix are replicated. No cross-core reduction.

The graded shape runs the _build_v7 kernel (~349 us/core measured vs a
~321 us DMA floor; the staged baseline was ~579 us).  Per core it is
HBM-bound: 12 x 8 MB of entries read + 8 MB written at ~320 GB/s.  Design
points, each worth 20-100 us on hardware:
  - 256-row blocks with 2 rows per SBUF partition so every DMA descriptor
    covers 8 KB contiguous DRAM (12 x 4 KB strided chunks cost ~27 us/iter)
  - the t dim is loaded as 12 single-t 1 MB tiles into an 18-deep buffer
    pool: an e-tile stays alive until the block's last matmul consumes it
    (~20 us after the block's final load), and fine tiles + deep pool keep
    the load queue from ever stalling on slot reuse
  - ent is DECLARED float32r in DRAM (same bits as fp32) so a plain HWDGE
    load feeds 1-cyc/row f32r matmuls (fp32 is 4 cyc/row) while the stats
    path reads the same bytes as exact fp32 via bitcast; SWDGE cast loads
    would bottleneck the Pool engine on descriptor generation
  - stats interleave the two row-chains per arriving tile; the softmax
    smalls run once per block on a u-major [128, 2T] layout (max-subtract
    on VectorE since ScalarE bias is per-partition only)
  - rinv = exp(-0.5*ln(ssq/D + eps)) on ScalarE; the act-table assigner is
    pinned (per-kernel instance) to `natural_log_exp_and_others` - stock
    first-fit flip-flops Ln->natural_log / Exp->exp_and_others and inserts
    a ~2.7 us table load TWICE PER BLOCK.  (A VectorE Newton rsqrt avoids
    tables entirely but loses 65 us: VectorE is the critical engine.)
  - TensorE: out = sum_t diag(ex_t) @ e_t into 8 single-bank PSUM
    accumulators; the PSUM->SBUF evacuation (activation Copy with the
    1/sum softmax scale folded in) and the output store are software-
    pipelined one block behind the matmuls
"""

import contextlib
import os
import sys

import numpy as np

sys.path.insert(0, "/opt/trn_rl_repo")

import concourse.bacc as bacc  # noqa: E402
import concourse.bass as bass  # noqa: E402
import concourse.tile as tile  # noqa: E402
from concourse import mybir  # noqa: E402
from concourse.bass_utils import run_bass_kernel_spmd  # noqa: E402

N_CORES = 8
P = 128
NORM_EPS = 1e-5

_kernel_cache = {}


def _pin_act_tables(nc, set_name="natural_log_exp_and_others"):
    """Make this kernel's act-table assignment use one set for everything.

    The stock first-fit assignment sends Ln to `natural_log` (no exp) and
    Square/Exp to `exp_and_others` (no ln), inserting a ~2.7us table load
    twice per block.  All four funcs we use (Square/Exp/Ln/Copy) live in
    `natural_log_exp_and_others`; emptying every other entry (positions
    preserved, so emitted set ids stay valid) makes first-fit land there
    for every activation -> exactly one load for the whole program.
    Instance-level override only; no global state touched.
    """
    import types

    import bass_rust as _brs

    from concourse.hw_specs import get_activation_tables

    def patched(self):
        has_act = any(
            isinstance(i, mybir.InstActivation)
            for b in self.main_func.blocks
            for i in b.instructions
        )
        if not has_act:
            return
        tables = [
            (name, fns if name == set_name else set())
            for name, fns in get_activation_tables(self.m.arch).items()
        ]
        _brs.insert_act_table_loads(self, tables)

    nc.insert_act_table_loads = types.MethodType(patched, nc)


def _declare_io(nc, T_act, R, D, bench_rep, ent_dt=None):
    f32 = mybir.dt.float32
    u32 = mybir.dt.uint32
    ent = nc.declare_dram_parameter("ent", [T_act, R, D], ent_dt or f32,
                                    isOutput=False)
    qv = nc.declare_dram_parameter("qv", [1, D], f32, isOutput=False)
    ident = nc.declare_dram_parameter("ident", [P, P], f32, isOutput=False)
    repc = None
    if bench_rep:
        repc = nc.declare_dram_parameter("repc", [1, 1], u32, isOutput=False)
    out = nc.declare_dram_parameter("out", [R, D], f32, isOutput=True)
    return ent, qv, ident, repc, out


def _build_v2(T_act, R, D, n_gq=0, dg_eng="gpsimd", ebufs=3, mode="full",
              mm_dtype="f32r", split_load=False, bench_rep=False, rep=1):
    """v2 builder.  T_act: active depth entries; R: rows/core; D: feature dim.

    n_gq: how many of the T_act qdot reductions run on GpSimd instead of
    VectorE.  dg_eng: engine for the diag(w) broadcast build.  mm_dtype:
    'f32r' (SWDGE rounds e to fp32r on load; PE at 1 cyc/row) | 'bf16'
    (SWDGE casts e+q' to bf16; 2x DVE qdot, half SBUF) | 'f32' (exact,
    PE at 4 cyc/row).  rep: run the body rep times in a static device
    loop (rep-delta wall-clock timing).  bench_rep: dynamic variant of
    the same via a [1,1] u32 'repc' input (hangs on HW — do not use).
    """
    f32 = mybir.dt.float32
    assert R % P == 0 and D % 512 == 0
    n_tiles = R // P

    nc = bacc.Bacc()
    ent_dt = mybir.dt.float32r if mm_dtype == "f32rd" else None
    ent, qv, ident, repc, out = _declare_io(nc, T_act, R, D, bench_rep,
                                            ent_dt=ent_dt)

    with tile.TileContext(nc) as tc:
        with (
            tc.tile_pool(name="singles", bufs=1) as singles,
            tc.tile_pool(name="ebuf", bufs=ebufs) as ebuf,
            tc.tile_pool(name="stats", bufs=2) as stats,
            tc.tile_pool(name="scr", bufs=1) as scr,
            tc.tile_pool(name="diag", bufs=2) as diagp,
            tc.tile_pool(name="outs", bufs=3) as outs,
            tc.tile_pool(name="psum", bufs=2, space="PSUM") as psump,
        ):
            qb_dt = mybir.dt.bfloat16 if mm_dtype == "bf16" else f32
            qb = singles.tile([P, D], qb_dt)
            nc.gpsimd.dma_start(out=qb, in_=qv[:, :].to_broadcast((P, D)))
            id_t = singles.tile([P, P], f32)
            nc.sync.dma_start(out=id_t, in_=ident[:, :])
            epsb = singles.tile([P, 1], f32)
            nc.vector.memset(epsb, float(NORM_EPS))

            if bench_rep:
                rt = singles.tile([1, 1], mybir.dt.uint32)
                nc.sync.dma_start(out=rt, in_=repc[:, :])
                _, (repv,) = nc.values_load_multi_w_load_instructions(
                    rt[0:1, 0:1], min_val=1, max_val=1 << 20
                )
                loop_ctx = tc.For_i(0, repv, 1)
            elif rep > 1:
                loop_ctx = tc.For_i(0, rep, 1)
            else:
                loop_ctx = contextlib.nullcontext()
            with loop_ctx:
                _v2_body(nc, T_act, D, n_tiles, n_gq, dg_eng, mode, mm_dtype,
                         split_load, ent, out, qb, id_t, epsb, ebuf, stats,
                         scr, diagp, outs, psump)

    nc.finalize()
    return nc


def _v2_body(nc, T_act, D, n_tiles, n_gq, dg_eng, mode, mm_dtype, split_load,
             ent, out, qb, id_t, epsb, ebuf, stats, scr, diagp, outs, psump):
    f32 = mybir.dt.float32
    AF = mybir.ActivationFunctionType
    OP = mybir.AluOpType
    nh = D // 512
    e_dt = {"f32": f32, "f32r": mybir.dt.float32r,
            "f32rd": mybir.dt.float32r, "bf16": mybir.dt.bfloat16}[mm_dtype]
    dg_dt = e_dt if mm_dtype != "f32" else f32

    for i in range(n_tiles):
        r0 = i * P
        if mode == "dmacontig":
            # BW probe: same bytes per tile but one contiguous 48KB chunk
            # per partition instead of 12 strided 4KB chunks.  Not a valid
            # compute layout.
            base = ent[:, 0:1, :]
            flat = bass.AP(
                tensor=base.tensor, offset=i * P * T_act * D,
                ap=[[T_act * D, P], [1, T_act * D]],
            )
            e2 = ebuf.tile([P, T_act * D], e_dt, tag="e")
            nc.sync.dma_start(out=e2, in_=flat)
            ob = outs.tile([P, D], f32)
            nc.scalar.copy(out=ob, in_=e2[:, 0:D].bitcast(f32))
            nc.scalar.dma_start(out=out[r0 : r0 + P, :], in_=ob)
            continue

        e = ebuf.tile([P, T_act, D], e_dt, tag="e")
        if mm_dtype in ("f32", "f32rd"):
            # HWDGE, no cast (f32rd: DRAM tensor itself is declared f32r —
            # same bits as f32, so the PE single-pass multiply just reads
            # unrounded fp32; fine at our tolerance)
            if split_load:
                th = T_act // 2
                nc.sync.dma_start(
                    out=e[:, 0:th, :],
                    in_=ent[0:th, r0 : r0 + P, :].rearrange("t s d -> s t d"),
                )
                nc.scalar.dma_start(
                    out=e[:, th:T_act, :],
                    in_=ent[th:T_act, r0 : r0 + P, :].rearrange(
                        "t s d -> s t d"),
                )
            else:
                nc.sync.dma_start(
                    out=e,
                    in_=ent[:, r0 : r0 + P, :].rearrange("t s d -> s t d"),
                )
            ec = e[:].bitcast(f32) if mm_dtype == "f32rd" else e
        else:
            # SWDGE casts f32 -> e_dt
            nc.gpsimd.dma_start(
                out=e, in_=ent[:, r0 : r0 + P, :].rearrange("t s d -> s t d")
            )
            ec = e[:].bitcast(f32) if mm_dtype == "f32r" else e

        if mode == "dmaonly":
            ob = outs.tile([P, D], f32)
            nc.scalar.copy(out=ob, in_=ec[:, 0, :])
            nc.scalar.dma_start(out=out[r0 : r0 + P, :], in_=ob)
            continue

        if mode == "nostats":
            # skip stats+softmax: uniform weights straight into the diag build
            ex = stats.tile([P, T_act], f32)
            nc.vector.memset(ex, 1.0 / T_act)
            rsum = stats.tile([P, 1], f32)
            nc.vector.memset(rsum, 1.0)
            _v2_mm(nc, T_act, nh, e, ex, rsum, id_t, dg_eng, dg_dt, diagp,
                   outs, psump, out, r0, D)
            continue

        ssq = stats.tile([P, T_act], f32)
        qd = stats.tile([P, T_act], f32)
        scr_dt = e_dt if mm_dtype == "bf16" else f32
        sq_scr = scr.tile([P, D], scr_dt, tag="sq")
        qd_scr = scr.tile([P, D], scr_dt, tag="qd")
        if n_gq:
            gq_scr = scr.tile([P, D], scr_dt, tag="gq")
        else:
            gq_scr = None
        for t in range(T_act):
            nc.scalar.activation(
                out=sq_scr,
                in_=ec[:, t, :],
                func=AF.Square,
                accum_out=ssq[:, t : t + 1],
            )
            on_gp = t >= T_act - n_gq
            (nc.gpsimd if on_gp else nc.vector).scalar_tensor_tensor(
                out=(gq_scr if on_gp else qd_scr),
                in0=ec[:, t, :],
                scalar=0.0,
                in1=qb,
                op0=OP.bypass,
                op1=OP.mult,
                accum_out=qd[:, t : t + 1],
            )

        # rinv = rsqrt(ssq/D + eps) = exp(-0.5 * ln(ssq/D + eps))
        lnm = stats.tile([P, T_act], f32)
        nc.scalar.activation(
            out=lnm, in_=ssq, func=AF.Ln, scale=1.0 / D, bias=epsb[:, 0:1]
        )
        rinv = stats.tile([P, T_act], f32)
        nc.scalar.activation(out=rinv, in_=lnm, func=AF.Exp, scale=-0.5)
        lg = stats.tile([P, T_act], f32)
        nc.vector.tensor_mul(lg, qd, rinv)

        # softmax over the free (t) axis; normalization deferred to the evac
        mx = stats.tile([P, 1], f32)
        nc.vector.tensor_reduce(
            out=mx, in_=lg, axis=mybir.AxisListType.X, op=OP.max
        )
        negm = stats.tile([P, 1], f32)
        nc.vector.tensor_scalar_mul(negm, mx, -1.0)
        ex = stats.tile([P, T_act], f32)
        sume = stats.tile([P, 1], f32)
        nc.scalar.activation(
            out=ex, in_=lg, func=AF.Exp, bias=negm, accum_out=sume
        )
        rsum = stats.tile([P, 1], f32)
        nc.vector.reciprocal(rsum, sume)

        if mode == "nomm":
            # skip the weighted sum: store a plain copy (tests stats path)
            ob = outs.tile([P, D], f32)
            nc.scalar.activation(out=ob, in_=ec[:, 0, :], func=AF.Copy,
                                 scale=rsum)
            nc.scalar.dma_start(out=out[r0 : r0 + P, :], in_=ob)
            continue

        _v2_mm(nc, T_act, nh, e, ex, rsum, id_t, dg_eng, dg_dt, diagp, outs,
               psump, out, r0, D)


def _v2_mm(nc, T_act, nh, e, ex, rsum, id_t, dg_eng, dg_dt, diagp, outs,
           psump, out, r0, D):
    f32 = mybir.dt.float32
    AF = mybir.ActivationFunctionType
    OP = mybir.AluOpType
    # dg[p, t, c] = id[p, c] * ex[p, t] via stride-0 broadcast inputs
    dg_all = diagp.tile([P, T_act, P], dg_dt, tag="dg")
    ida = id_t[:, :]
    wa = ex[:, 0:T_act]
    idb = bass.AP(tensor=ida.tensor, offset=ida.offset,
                  ap=[ida.ap[0], [0, T_act], ida.ap[1]])
    wb = bass.AP(tensor=wa.tensor, offset=wa.offset,
                 ap=[wa.ap[0], wa.ap[1], [0, P]])
    dg_builder = nc.gpsimd if dg_eng == "gpsimd" else nc.vector
    dg_builder.tensor_tensor(out=dg_all, in0=idb, in1=wb, op=OP.mult)

    po = psump.tile([P, D], f32)
    for t in range(T_act):
        lhsT = dg_all[:, t, :]
        for h in range(nh):
            cs = slice(h * 512, (h + 1) * 512)
            nc.tensor.matmul(
                po[:, cs],
                lhsT=lhsT,
                rhs=e[:, t, cs],
                start=(t == 0),
                stop=(t == T_act - 1),
            )

    ob = outs.tile([P, D], f32)
    nc.scalar.activation(out=ob, in_=po, func=AF.Copy, scale=rsum)
    nc.scalar.dma_start(out=out[r0 : r0 + P, :], in_=ob)


# ---------------------------------------------------------------------------
# v4: 256-row blocks, 2 rows per partition -> 8 KB-contiguous DMA descriptors
# (vs 4 KB in v3), t-dim split into two tile-group loads to keep SBUF bounded.
# ---------------------------------------------------------------------------


def _build_v4(T_act, R, D, dg_eng="vector", ebufs=6, n_tg=4, n_gq=0,
              mode="full", split_load=False, rep=1):
    """256-row blocks (2 rows/partition -> 8 KB-contiguous DMA descriptors);
    t dim loaded as n_tg separate tile-groups so the e-buffer pool rotates
    at fine granularity (an e tile stays alive until the weighted-sum matmul
    consumes it ~30-40 us after stats start; small tiles + many bufs keep
    the load pipe from stalling on slot reuse)."""
    f32 = mybir.dt.float32
    f32r = mybir.dt.float32r
    AF = mybir.ActivationFunctionType
    OP = mybir.AluOpType
    U = 2  # rows per partition
    RB = P * U  # rows per block
    assert R % RB == 0 and D % 512 == 0 and T_act >= 2
    n_blocks = R // RB
    # t-group boundaries, e.g. T=12, n_tg=4 -> [(0,3),(3,6),(6,9),(9,12)]
    n_tg = min(n_tg, T_act)
    gsz = (T_act + n_tg - 1) // n_tg
    tg = []
    t0 = 0
    while t0 < T_act:
        tg.append((t0, min(t0 + gsz, T_act)))
        t0 += gsz
    nh = D // 512

    nc = bacc.Bacc()
    ent, qv, ident, repc, out = _declare_io(nc, T_act, R, D, False,
                                            ent_dt=f32r)

    with tile.TileContext(nc) as tc:
        with (
            tc.tile_pool(name="singles", bufs=1) as singles,
            tc.tile_pool(name="ebuf", bufs=ebufs) as ebuf,
            tc.tile_pool(name="stats", bufs=2) as stats,
            tc.tile_pool(name="scr", bufs=1) as scr,
            tc.tile_pool(name="diag", bufs=2) as diagp,
            tc.tile_pool(name="outs", bufs=2) as outs,
            tc.tile_pool(name="psum", bufs=2, space="PSUM") as psump,
        ):
            qb = singles.tile([P, D], f32)
            nc.gpsimd.dma_start(out=qb, in_=qv[:, :].to_broadcast((P, D)))
            id_t = singles.tile([P, P], f32)
            nc.sync.dma_start(out=id_t, in_=ident[:, :])
            epsb = singles.tile([P, 1], f32)
            nc.vector.memset(epsb, float(NORM_EPS))

            loop_ctx = tc.For_i(0, rep, 1) if rep > 1 else \
                contextlib.nullcontext()
            with loop_ctx:
                for i in range(n_blocks):
                    r0 = i * RB
                    egs = []
                    for gi, (g0, g1) in enumerate(tg):
                        eg = ebuf.tile([P, g1 - g0, U, D], f32r, tag="e")
                        eng = nc.scalar if (split_load and gi % 2) else \
                            nc.sync
                        eng.dma_start(
                            out=eg,
                            in_=ent[g0:g1, r0 : r0 + RB, :].rearrange(
                                "t (s u) d -> s t u d", u=U),
                        )
                        egs.append(eg)

                    def et(t, u, cs=slice(None)):
                        gi = t // gsz
                        return egs[gi][:, t - tg[gi][0], u, cs]

                    ob = outs.tile([P, U, D], f32)

                    if mode == "dmaonly":
                        nc.scalar.copy(out=ob[:, 0, :],
                                       in_=et(0, 0).bitcast(f32))
                        nc.scalar.copy(out=ob[:, 1, :],
                                       in_=et(0, 1).bitcast(f32))
                        nc.scalar.dma_start(
                            out=out[r0 : r0 + RB, :].rearrange(
                                "(s u) d -> s u d", u=U),
                            in_=ob,
                        )
                        continue

                    sq_scr = scr.tile([P, D], f32, tag="sq")
                    qd_scr = scr.tile([P, D], f32, tag="qd")
                    if n_gq:
                        gq_scr = scr.tile([P, D], f32, tag="gq")
                    for u in range(U):
                        if mode == "nostats":
                            ex = stats.tile([P, T_act], f32)
                            nc.vector.memset(ex, 1.0 / T_act)
                            rsum = stats.tile([P, 1], f32)
                            nc.vector.memset(rsum, 1.0)
                            _v4_mm(nc, T_act, nh, et, u, ex, rsum, id_t,
                                   dg_eng, diagp, psump, ob)
                            continue
                        ssq = stats.tile([P, T_act], f32)
                        qd = stats.tile([P, T_act], f32)
                        for t in range(T_act):
                            ef = et(t, u).bitcast(f32)
                            nc.scalar.activation(
                                out=sq_scr, in_=ef, func=AF.Square,
                                accum_out=ssq[:, t : t + 1],
                            )
                            on_gp = t >= T_act - n_gq
                            (nc.gpsimd if on_gp else
                             nc.vector).scalar_tensor_tensor(
                                out=(gq_scr if on_gp else qd_scr),
                                in0=ef, scalar=0.0, in1=qb,
                                op0=OP.bypass, op1=OP.mult,
                                accum_out=qd[:, t : t + 1],
                            )

                        lnm = stats.tile([P, T_act], f32)
                        nc.scalar.activation(
                            out=lnm, in_=ssq, func=AF.Ln, scale=1.0 / D,
                            bias=epsb[:, 0:1],
                        )
                        rinv = stats.tile([P, T_act], f32)
                        nc.scalar.activation(out=rinv, in_=lnm, func=AF.Exp,
                                             scale=-0.5)
                        lg = stats.tile([P, T_act], f32)
                        nc.vector.tensor_mul(lg, qd, rinv)
                        mx = stats.tile([P, 1], f32)
                        nc.vector.tensor_reduce(
                            out=mx, in_=lg, axis=mybir.AxisListType.X,
                            op=OP.max,
                        )
                        negm = stats.tile([P, 1], f32)
                        nc.vector.tensor_scalar_mul(negm, mx, -1.0)
                        ex = stats.tile([P, T_act], f32)
                        sume = stats.tile([P, 1], f32)
                        nc.scalar.activation(
                            out=ex, in_=lg, func=AF.Exp, bias=negm,
                            accum_out=sume,
                        )
                        rsum = stats.tile([P, 1], f32)
                        nc.vector.reciprocal(rsum, sume)

                        if mode == "nomm":
                            nc.scalar.activation(out=ob[:, u, :],
                                                 in_=et(0, u).bitcast(f32),
                                                 func=AF.Copy, scale=rsum)
                            continue

                        _v4_mm(nc, T_act, nh, et, u, ex, rsum, id_t, dg_eng,
                               diagp, psump, ob)

                    nc.scalar.dma_start(
                        out=out[r0 : r0 + RB, :].rearrange(
                            "(s u) d -> s u d", u=U),
                        in_=ob,
                    )

    nc.finalize()
    return nc


def _v4_mm(nc, T_act, nh, et, u, ex, rsum, id_t, dg_eng, diagp, psump, ob):
    f32 = mybir.dt.float32
    f32r = mybir.dt.float32r
    AF = mybir.ActivationFunctionType
    OP = mybir.AluOpType
    dg_all = diagp.tile([P, T_act, P], f32r, tag="dg")
    ida = id_t[:, :]
    wa = ex[:, 0:T_act]
    idb = bass.AP(tensor=ida.tensor, offset=ida.offset,
                  ap=[ida.ap[0], [0, T_act], ida.ap[1]])
    wb = bass.AP(tensor=wa.tensor, offset=wa.offset,
                 ap=[wa.ap[0], wa.ap[1], [0, P]])
    dg_builder = nc.gpsimd if dg_eng == "gpsimd" else nc.vector
    dg_builder.tensor_tensor(out=dg_all, in0=idb, in1=wb, op=OP.mult)

    po = psump.tile([P, nh * 512], f32)
    for t in range(T_act):
        lhsT = dg_all[:, t, :]
        for h in range(nh):
            cs = slice(h * 512, (h + 1) * 512)
            nc.tensor.matmul(
                po[:, cs], lhsT=lhsT, rhs=et(t, u, cs),
                start=(t == 0), stop=(t == T_act - 1),
            )

    nc.scalar.activation(out=ob[:, u, :], in_=po, func=AF.Copy, scale=rsum)


# ---------------------------------------------------------------------------
# v5 = v4 + one-block software-pipelined PSUM evacuation.  ScalarE's queue is
# strict FIFO: with the evac emitted right after a block's matmuls, ScalarE
# sits idle waiting for the softmax->diag->matmul chain before it can issue
# the evac, stalling the NEXT block's Square ops behind it.  Deferring each
# block's evac+store into the following block (PSUM: 8 single-bank
# accumulators, two blocks in flight) removes the stall.
# ---------------------------------------------------------------------------


def _build_v5(T_act, R, D, n_tg=4, ebufs=6, mode="full", rep=1):
    f32 = mybir.dt.float32
    f32r = mybir.dt.float32r
    AF = mybir.ActivationFunctionType
    OP = mybir.AluOpType
    U = 2  # rows per partition
    RB = P * U
    assert R % RB == 0 and D == 1024 and T_act >= 2
    n_blocks = R // RB
    n_tg = min(n_tg, T_act)
    gsz = (T_act + n_tg - 1) // n_tg
    tg = []
    t0 = 0
    while t0 < T_act:
        tg.append((t0, min(t0 + gsz, T_act)))
        t0 += gsz
    nh = D // 512

    nc = bacc.Bacc()
    ent, qv, ident, repc, out = _declare_io(nc, T_act, R, D, False,
                                            ent_dt=f32r)

    with tile.TileContext(nc) as tc:
        with (
            tc.tile_pool(name="singles", bufs=1) as singles,
            tc.tile_pool(name="ebuf", bufs=ebufs) as ebuf,
            tc.tile_pool(name="stats", bufs=2) as stats,
            tc.tile_pool(name="scr", bufs=1) as scr,
            tc.tile_pool(name="diag", bufs=2) as diagp,
            tc.tile_pool(name="outs", bufs=2) as outs,
            tc.tile_pool(name="psum", bufs=2, space="PSUM") as psump,
        ):
            qb = singles.tile([P, D], f32)
            nc.gpsimd.dma_start(out=qb, in_=qv[:, :].to_broadcast((P, D)))
            id_t = singles.tile([P, P], f32)
            nc.sync.dma_start(out=id_t, in_=ident[:, :])
            epsb = singles.tile([P, 1], f32)
            nc.vector.memset(epsb, float(NORM_EPS))

            def flush(pending):
                po_list, rsums, r0p = pending
                ob = outs.tile([P, U, D], f32, tag="ob")
                for u, h, po in po_list:
                    nc.scalar.activation(
                        out=ob[:, u, h * 512 : (h + 1) * 512], in_=po,
                        func=AF.Copy, scale=rsums[u],
                    )
                nc.scalar.dma_start(
                    out=out[r0p : r0p + RB, :].rearrange(
                        "(s u) d -> s u d", u=U),
                    in_=ob,
                )

            loop_ctx = tc.For_i(0, rep, 1) if rep > 1 else \
                contextlib.nullcontext()
            with loop_ctx:
                pending = None
                for i in range(n_blocks):
                    r0 = i * RB
                    egs = []
                    for gi, (g0, g1) in enumerate(tg):
                        eg = ebuf.tile([P, g1 - g0, U, D], f32r, tag="e")
                        nc.sync.dma_start(
                            out=eg,
                            in_=ent[g0:g1, r0 : r0 + RB, :].rearrange(
                                "t (s u) d -> s t u d", u=U),
                        )
                        egs.append(eg)

                    def et(t, u, cs=slice(None)):
                        gi = t // gsz
                        return egs[gi][:, t - tg[gi][0], u, cs]

                    sq_scr = scr.tile([P, D], f32, tag="sq")
                    qd_scr = scr.tile([P, D], f32, tag="qd")
                    this_po = []
                    this_rsum = {}
                    for u in range(U):
                        ssq = stats.tile([P, T_act], f32)
                        qd = stats.tile([P, T_act], f32)
                        for t in range(T_act):
                            ef = et(t, u).bitcast(f32)
                            nc.scalar.activation(
                                out=sq_scr, in_=ef, func=AF.Square,
                                accum_out=ssq[:, t : t + 1],
                            )
                            nc.vector.scalar_tensor_tensor(
                                out=qd_scr, in0=ef, scalar=0.0, in1=qb,
                                op0=OP.bypass, op1=OP.mult,
                                accum_out=qd[:, t : t + 1],
                            )

                        lnm = stats.tile([P, T_act], f32)
                        nc.scalar.activation(
                            out=lnm, in_=ssq, func=AF.Ln, scale=1.0 / D,
                            bias=epsb[:, 0:1],
                        )
                        rinv = stats.tile([P, T_act], f32)
                        nc.scalar.activation(out=rinv, in_=lnm, func=AF.Exp,
                                             scale=-0.5)
                        lg = stats.tile([P, T_act], f32)
                        nc.vector.tensor_mul(lg, qd, rinv)
                        mx = stats.tile([P, 1], f32)
                        nc.vector.tensor_reduce(
                            out=mx, in_=lg, axis=mybir.AxisListType.X,
                            op=OP.max,
                        )
                        negm = stats.tile([P, 1], f32)
                        nc.vector.tensor_scalar_mul(negm, mx, -1.0)
                        ex = stats.tile([P, T_act], f32)
                        sume = stats.tile([P, 1], f32)
                        nc.scalar.activation(
                            out=ex, in_=lg, func=AF.Exp, bias=negm,
                            accum_out=sume,
                        )
                        rsum = stats.tile([P, 1], f32, tag=f"rs{u}")
                        nc.vector.reciprocal(rsum, sume)
                        this_rsum[u] = rsum

                        if mode == "nodep":
                            # same work, severed dependency: dg consumes a
                            # memset instead of the softmax output
                            ex = stats.tile([P, T_act], f32, tag=f"exm{u}")
                            nc.vector.memset(ex, 1.0 / T_act)

                        dg_all = diagp.tile([P, T_act, P], f32r, tag="dg")
                        ida = id_t[:, :]
                        wa = ex[:, 0:T_act]
                        idb = bass.AP(tensor=ida.tensor, offset=ida.offset,
                                      ap=[ida.ap[0], [0, T_act], ida.ap[1]])
                        wb = bass.AP(tensor=wa.tensor, offset=wa.offset,
                                     ap=[wa.ap[0], wa.ap[1], [0, P]])
                        nc.vector.tensor_tensor(out=dg_all, in0=idb, in1=wb,
                                                op=OP.mult)

                        pos = []
                        for h in range(nh):
                            po = psump.tile([P, 512], f32, tag=f"po{u}{h}")
                            pos.append(po)
                        for t in range(T_act):
                            lhsT = dg_all[:, t, :]
                            for h in range(nh):
                                cs = slice(h * 512, (h + 1) * 512)
                                nc.tensor.matmul(
                                    pos[h], lhsT=lhsT, rhs=et(t, u, cs),
                                    start=(t == 0), stop=(t == T_act - 1),
                                )
                        for h in range(nh):
                            this_po.append((u, h, pos[h]))

                    if pending is not None:
                        flush(pending)
                    pending = (this_po, this_rsum, r0)
                flush(pending)

    nc.finalize()
    return nc


# ---------------------------------------------------------------------------
# v6 = v5 + stats interleaved across the two row-chains (u) per arriving
# tile-group, finer 2-t tile-groups with a deep e-buffer pool.  The e-tiles
# of a block stay alive until the block's LAST matmul; the post-load tail
# (remaining qdots -> softmax -> diag -> matmul) is what sets tile lifetime,
# so interleaving u keeps the tail short and 10 small buffers cover it.
# ---------------------------------------------------------------------------


def _build_v6(T_act, R, D, n_tg=6, ebufs=10, rep=1):
    f32 = mybir.dt.float32
    f32r = mybir.dt.float32r
    bf16 = mybir.dt.bfloat16
    AF = mybir.ActivationFunctionType
    OP = mybir.AluOpType
    U = 2  # rows per partition
    RB = P * U
    assert R % RB == 0 and D == 1024 and T_act >= 2
    n_blocks = R // RB
    n_tg = min(n_tg, T_act)
    gsz = (T_act + n_tg - 1) // n_tg
    tg = []
    t0 = 0
    while t0 < T_act:
        tg.append((t0, min(t0 + gsz, T_act)))
        t0 += gsz
    nh = D // 512

    nc = bacc.Bacc()
    ent, qv, ident, repc, out = _declare_io(nc, T_act, R, D, False,
                                            ent_dt=f32r)

    with tile.TileContext(nc) as tc:
        with (
            tc.tile_pool(name="singles", bufs=1) as singles,
            tc.tile_pool(name="ebuf", bufs=ebufs) as ebuf,
            tc.tile_pool(name="stats", bufs=2) as stats,
            tc.tile_pool(name="scr", bufs=1) as scr,
            tc.tile_pool(name="diag", bufs=2) as diagp,
            tc.tile_pool(name="outs", bufs=2) as outs,
            tc.tile_pool(name="psum", bufs=2, space="PSUM") as psump,
        ):
            qb = singles.tile([P, D], f32)
            nc.gpsimd.dma_start(out=qb, in_=qv[:, :].to_broadcast((P, D)))
            id_t = singles.tile([P, P], f32)
            nc.sync.dma_start(out=id_t, in_=ident[:, :])
            epsb = singles.tile([P, 1], f32)
            nc.vector.memset(epsb, float(NORM_EPS))

            def flush(pending):
                po_list, rsums, r0p = pending
                ob = outs.tile([P, U, D], f32, tag="ob")
                for u, h, po in po_list:
                    nc.scalar.activation(
                        out=ob[:, u, h * 512 : (h + 1) * 512], in_=po,
                        func=AF.Copy, scale=rsums[u],
                    )
                nc.scalar.dma_start(
                    out=out[r0p : r0p + RB, :].rearrange(
                        "(s u) d -> s u d", u=U),
                    in_=ob,
                )

            loop_ctx = tc.For_i(0, rep, 1) if rep > 1 else \
                contextlib.nullcontext()
            with loop_ctx:
                pending = None
                for i in range(n_blocks):
                    r0 = i * RB
                    egs = []
                    for gi, (g0, g1) in enumerate(tg):
                        eg = ebuf.tile([P, g1 - g0, U, D], f32r, tag="e")
                        nc.sync.dma_start(
                            out=eg,
                            in_=ent[g0:g1, r0 : r0 + RB, :].rearrange(
                                "t (s u) d -> s t u d", u=U),
                        )
                        egs.append(eg)

                    def et(t, u, cs=slice(None)):
                        gi = t // gsz
                        return egs[gi][:, t - tg[gi][0], u, cs]

                    sq_scr = scr.tile([P, D], bf16, tag="sq")
                    qd_scr = scr.tile([P, D], bf16, tag="qd")
                    ssqs = [stats.tile([P, T_act], f32, tag=f"ssq{u}",
                                       name=f"ssq{u}") for u in range(U)]
                    qds = [stats.tile([P, T_act], f32, tag=f"qd{u}",
                                      name=f"qd{u}") for u in range(U)]
                    # stats interleaved in tile-arrival order
                    for t in range(T_act):
                        for u in range(U):
                            ef = et(t, u).bitcast(f32)
                            nc.scalar.activation(
                                out=sq_scr, in_=ef, func=AF.Square,
                                accum_out=ssqs[u][:, t : t + 1],
                            )
                            nc.vector.scalar_tensor_tensor(
                                out=qd_scr, in0=ef, scalar=0.0, in1=qb,
                                op0=OP.bypass, op1=OP.mult,
                                accum_out=qds[u][:, t : t + 1],
                            )

                    this_po = []
                    this_rsum = {}
                    for u in range(U):
                        lnm = stats.tile([P, T_act], f32)
                        nc.scalar.activation(
                            out=lnm, in_=ssqs[u], func=AF.Ln, scale=1.0 / D,
                            bias=epsb[:, 0:1],
                        )
                        rinv = stats.tile([P, T_act], f32)
                        nc.scalar.activation(out=rinv, in_=lnm, func=AF.Exp,
                                             scale=-0.5)
                        lg = stats.tile([P, T_act], f32)
                        nc.vector.tensor_mul(lg, qds[u], rinv)
                        mx = stats.tile([P, 1], f32)
                        nc.vector.tensor_reduce(
                            out=mx, in_=lg, axis=mybir.AxisListType.X,
                            op=OP.max,
                        )
                        negm = stats.tile([P, 1], f32)
                        nc.vector.tensor_scalar_mul(negm, mx, -1.0)
                        ex = stats.tile([P, T_act], f32)
                        sume = stats.tile([P, 1], f32)
                        nc.scalar.activation(
                            out=ex, in_=lg, func=AF.Exp, bias=negm,
                            accum_out=sume,
                        )
                        rsum = stats.tile([P, 1], f32, tag=f"rs{u}")
                        nc.vector.reciprocal(rsum, sume)
                        this_rsum[u] = rsum

                        dg_all = diagp.tile([P, T_act, P], f32r, tag="dg")
                        ida = id_t[:, :]
                        wa = ex[:, 0:T_act]
                        idb = bass.AP(tensor=ida.tensor, offset=ida.offset,
                                      ap=[ida.ap[0], [0, T_act], ida.ap[1]])
                        wb = bass.AP(tensor=wa.tensor, offset=wa.offset,
                                     ap=[wa.ap[0], wa.ap[1], [0, P]])
                        nc.vector.tensor_tensor(out=dg_all, in0=idb, in1=wb,
                                                op=OP.mult)

                        pos = []
                        for h in range(nh):
                            po = psump.tile([P, 512], f32, tag=f"po{u}{h}")
                            pos.append(po)
                        for t in range(T_act):
                            lhsT = dg_all[:, t, :]
                            for h in range(nh):
                                cs = slice(h * 512, (h + 1) * 512)
                                nc.tensor.matmul(
                                    pos[h], lhsT=lhsT, rhs=et(t, u, cs),
                                    start=(t == 0), stop=(t == T_act - 1),
                                )
                        for h in range(nh):
                            this_po.append((u, h, pos[h]))

                    if pending is not None:
                        flush(pending)
                    pending = (this_po, this_rsum, r0)
                flush(pending)

    nc.finalize()
    return nc


# ---------------------------------------------------------------------------
# v7 = v6 with the per-u softmax smalls merged into single wide ops over a
# u-major [P, U*T] stats layout (one Ln, one Exp, one mul, one strided max-
# reduce, one broadcast subtract, one strided sum-reduce, one reciprocal per
# block instead of 2x each) — fewer small-op overheads and fewer ScalarE<->
# VectorE ping-pongs in the post-load tail.
# ---------------------------------------------------------------------------


def _build_v7(T_act, R, D, n_tg=6, ebufs=10, dg_eng="vector", pin_tables=True,
              obufs=2, dbufs=2, sbufs=2, rep=1):
    f32 = mybir.dt.float32
    f32r = mybir.dt.float32r
    bf16 = mybir.dt.bfloat16
    AF = mybir.ActivationFunctionType
    OP = mybir.AluOpType
    U = 2
    RB = P * U
    assert R % RB == 0 and D == 1024 and T_act >= 2
    n_blocks = R // RB
    n_tg = min(n_tg, T_act)
    gsz = (T_act + n_tg - 1) // n_tg
    tg = []
    t0 = 0
    while t0 < T_act:
        tg.append((t0, min(t0 + gsz, T_act)))
        t0 += gsz
    nh = D // 512

    nc = bacc.Bacc()
    if pin_tables:
        _pin_act_tables(nc)
    ent, qv, ident, repc, out = _declare_io(nc, T_act, R, D, False,
                                            ent_dt=f32r)

    with tile.TileContext(nc) as tc:
        with (
            tc.tile_pool(name="singles", bufs=1) as singles,
            tc.tile_pool(name="ebuf", bufs=ebufs) as ebuf,
            tc.tile_pool(name="stats", bufs=sbufs) as stats,
            tc.tile_pool(name="scr", bufs=1) as scr,
            tc.tile_pool(name="diag", bufs=dbufs) as diagp,
            tc.tile_pool(name="outs", bufs=obufs) as outs,
            tc.tile_pool(name="psum", bufs=2, space="PSUM") as psump,
        ):
            qb = singles.tile([P, D], f32)
            nc.gpsimd.dma_start(out=qb, in_=qv[:, :].to_broadcast((P, D)))
            id_t = singles.tile([P, P], f32)
            nc.sync.dma_start(out=id_t, in_=ident[:, :])
            epsb = singles.tile([P, 1], f32)
            nc.vector.memset(epsb, float(NORM_EPS))

            def flush(pending):
                po_list, rsum_p, r0p = pending
                ob = outs.tile([P, U, D], f32, tag="ob")
                for u, h, po in po_list:
                    nc.scalar.activation(
                        out=ob[:, u, h * 512 : (h + 1) * 512], in_=po,
                        func=AF.Copy, scale=rsum_p[:, u : u + 1],
                    )
                nc.scalar.dma_start(
                    out=out[r0p : r0p + RB, :].rearrange(
                        "(s u) d -> s u d", u=U),
                    in_=ob,
                )

            loop_ctx = tc.For_i(0, rep, 1) if rep > 1 else \
                contextlib.nullcontext()
            with loop_ctx:
                pending = None
                for i in range(n_blocks):
                    r0 = i * RB
                    egs = []
                    for gi, (g0, g1) in enumerate(tg):
                        eg = ebuf.tile([P, g1 - g0, U, D], f32r, tag="e")
                        nc.sync.dma_start(
                            out=eg,
                            in_=ent[g0:g1, r0 : r0 + RB, :].rearrange(
                                "t (s u) d -> s t u d", u=U),
                        )
                        egs.append(eg)

                    def et(t, u, cs=slice(None)):
                        gi = t // gsz
                        return egs[gi][:, t - tg[gi][0], u, cs]

                    sq_scr = scr.tile([P, D], bf16, tag="sq")
                    qd_scr = scr.tile([P, D], bf16, tag="qd")
                    # u-major stats: column u*T_act + t
                    ssq = stats.tile([P, U * T_act], f32, tag="ssq")
                    qd = stats.tile([P, U * T_act], f32, tag="qd")
                    for t in range(T_act):
                        for u in range(U):
                            c = u * T_act + t
                            ef = et(t, u).bitcast(f32)
                            nc.scalar.activation(
                                out=sq_scr, in_=ef, func=AF.Square,
                                accum_out=ssq[:, c : c + 1],
                            )
                            nc.vector.scalar_tensor_tensor(
                                out=qd_scr, in0=ef, scalar=0.0, in1=qb,
                                op0=OP.bypass, op1=OP.mult,
                                accum_out=qd[:, c : c + 1],
                            )

                    # merged softmax smalls over [P, U*T]
                    lnm = stats.tile([P, U * T_act], f32)
                    nc.scalar.activation(
                        out=lnm, in_=ssq, func=AF.Ln, scale=1.0 / D,
                        bias=epsb[:, 0:1],
                    )
                    rinv = stats.tile([P, U * T_act], f32)
                    nc.scalar.activation(out=rinv, in_=lnm, func=AF.Exp,
                                         scale=-0.5)
                    lg = stats.tile([P, U * T_act], f32)
                    nc.vector.tensor_mul(lg, qd, rinv)
                    lg3 = lg[:].rearrange("p (u t) -> p u t", u=U)
                    mx = stats.tile([P, U], f32)
                    nc.vector.tensor_reduce(
                        out=mx, in_=lg3, axis=mybir.AxisListType.X, op=OP.max
                    )
                    # lgc = lg - mx  (mx broadcast over t via stride-0 AP)
                    mxa = mx[:, :]
                    mxb = bass.AP(tensor=mxa.tensor, offset=mxa.offset,
                                  ap=[mxa.ap[0], mxa.ap[1], [0, T_act]])
                    lgc = stats.tile([P, U * T_act], f32, tag="lgc")
                    nc.vector.tensor_tensor(
                        out=lgc[:].rearrange("p (u t) -> p u t", u=U),
                        in0=lg3, in1=mxb, op=OP.subtract,
                    )
                    ex = stats.tile([P, U * T_act], f32, tag="ex")
                    nc.scalar.activation(out=ex, in_=lgc, func=AF.Exp)
                    sume = stats.tile([P, U], f32)
                    nc.vector.tensor_reduce(
                        out=sume,
                        in_=ex[:].rearrange("p (u t) -> p u t", u=U),
                        axis=mybir.AxisListType.X, op=OP.add,
                    )
                    rsum = stats.tile([P, U], f32, tag="rsum")
                    nc.vector.reciprocal(rsum, sume)

                    this_po = []
                    for u in range(U):
                        dg_all = diagp.tile([P, T_act, P], f32r, tag="dg")
                        ida = id_t[:, :]
                        wa = ex[:, u * T_act : (u + 1) * T_act]
                        idb = bass.AP(tensor=ida.tensor, offset=ida.offset,
                                      ap=[ida.ap[0], [0, T_act], ida.ap[1]])
                        wb = bass.AP(tensor=wa.tensor, offset=wa.offset,
                                     ap=[wa.ap[0], wa.ap[1], [0, P]])
                        dg_builder = (nc.gpsimd if dg_eng == "gpsimd"
                                      else nc.vector)
                        dg_builder.tensor_tensor(out=dg_all, in0=idb, in1=wb,
                                                 op=OP.mult)

                        pos = []
                        for h in range(nh):
                            po = psump.tile([P, 512], f32, tag=f"po{u}{h}")
                            pos.append(po)
                        for t in range(T_act):
                            lhsT = dg_all[:, t, :]
                            for h in range(nh):
                                cs = slice(h * 512, (h + 1) * 512)
                                nc.tensor.matmul(
                                    pos[h], lhsT=lhsT, rhs=et(t, u, cs),
                                    start=(t == 0), stop=(t == T_act - 1),
                                )
                        for h in range(nh):
                            this_po.append((u, h, pos[h]))

                    if pending is not None:
                        flush(pending)
                    pending = (this_po, rsum, r0)
                flush(pending)

    nc.finalize()
    return nc


# ---------------------------------------------------------------------------
# v8 = v7 with the Ln-based rsqrt replaced by the integer-seed Newton rsqrt
# on VectorE.  Using Ln makes the act-table-set assigner flip-flop between
# two table sets (Square/Exp -> exp_and_others, Ln -> natural_log_...),
# inserting a ~2.7 us InstLoadActFuncSet TWICE PER BLOCK (~42 us/iter, and
# it sits in the softmax dependency tail).  With only Square/Exp/Copy the
# whole kernel needs ONE table load.
# ---------------------------------------------------------------------------


def _build_v8(T_act, R, D, n_tg=6, ebufs=10, newton=2, rep=1):
    f32 = mybir.dt.float32
    f32r = mybir.dt.float32r
    u32 = mybir.dt.uint32
    bf16 = mybir.dt.bfloat16
    AF = mybir.ActivationFunctionType
    OP = mybir.AluOpType
    U = 2
    RB = P * U
    assert R % RB == 0 and D == 1024 and T_act >= 2
    n_blocks = R // RB
    n_tg = min(n_tg, T_act)
    gsz = (T_act + n_tg - 1) // n_tg
    tg = []
    t0 = 0
    while t0 < T_act:
        tg.append((t0, min(t0 + gsz, T_act)))
        t0 += gsz
    nh = D // 512
    UT = U * T_act

    nc = bacc.Bacc()
    ent, qv, ident, repc, out = _declare_io(nc, T_act, R, D, False,
                                            ent_dt=f32r)

    with tile.TileContext(nc) as tc:
        with (
            tc.tile_pool(name="singles", bufs=1) as singles,
            tc.tile_pool(name="ebuf", bufs=ebufs) as ebuf,
            tc.tile_pool(name="stats", bufs=2) as stats,
            tc.tile_pool(name="scr", bufs=1) as scr,
            tc.tile_pool(name="diag", bufs=2) as diagp,
            tc.tile_pool(name="outs", bufs=2) as outs,
            tc.tile_pool(name="psum", bufs=2, space="PSUM") as psump,
        ):
            qb = singles.tile([P, D], f32)
            nc.gpsimd.dma_start(out=qb, in_=qv[:, :].to_broadcast((P, D)))
            id_t = singles.tile([P, P], f32)
            nc.sync.dma_start(out=id_t, in_=ident[:, :])

            def flush(pending):
                po_list, rsum_p, r0p = pending
                ob = outs.tile([P, U, D], f32, tag="ob")
                for u, h, po in po_list:
                    nc.scalar.activation(
                        out=ob[:, u, h * 512 : (h + 1) * 512], in_=po,
                        func=AF.Copy, scale=rsum_p[:, u : u + 1],
                    )
                nc.scalar.dma_start(
                    out=out[r0p : r0p + RB, :].rearrange(
                        "(s u) d -> s u d", u=U),
                    in_=ob,
                )

            loop_ctx = tc.For_i(0, rep, 1) if rep > 1 else \
                contextlib.nullcontext()
            with loop_ctx:
                pending = None
                for i in range(n_blocks):
                    r0 = i * RB
                    egs = []
                    for gi, (g0, g1) in enumerate(tg):
                        eg = ebuf.tile([P, g1 - g0, U, D], f32r, tag="e")
                        nc.sync.dma_start(
                            out=eg,
                            in_=ent[g0:g1, r0 : r0 + RB, :].rearrange(
                                "t (s u) d -> s t u d", u=U),
                        )
                        egs.append(eg)

                    def et(t, u, cs=slice(None)):
                        gi = t // gsz
                        return egs[gi][:, t - tg[gi][0], u, cs]

                    sq_scr = scr.tile([P, D], bf16, tag="sq")
                    qd_scr = scr.tile([P, D], bf16, tag="qd")
                    # u-major stats: column u*T_act + t
                    ssq = stats.tile([P, UT], f32, tag="ssq")
                    qd = stats.tile([P, UT], f32, tag="qd")
                    for t in range(T_act):
                        for u in range(U):
                            c = u * T_act + t
                            ef = et(t, u).bitcast(f32)
                            nc.scalar.activation(
                                out=sq_scr, in_=ef, func=AF.Square,
                                accum_out=ssq[:, c : c + 1],
                            )
                            nc.vector.scalar_tensor_tensor(
                                out=qd_scr, in0=ef, scalar=0.0, in1=qb,
                                op0=OP.bypass, op1=OP.mult,
                                accum_out=qd[:, c : c + 1],
                            )

                    # rinv = rsqrt(ssq/D + eps): integer-seed Newton on DVE
                    ms = stats.tile([P, UT], f32)
                    nc.vector.tensor_scalar(
                        out=ms, in0=ssq, scalar1=1.0 / D,
                        scalar2=float(NORM_EPS), op0=OP.mult, op1=OP.add,
                    )
                    sh = stats.tile([P, UT], u32)
                    nc.vector.tensor_scalar(
                        out=sh, in0=ms[:].bitcast(u32), scalar1=1,
                        scalar2=None, op0=OP.logical_shift_right,
                    )
                    shf = stats.tile([P, UT], f32)
                    nc.vector.tensor_copy(shf, sh)
                    nc.vector.tensor_scalar(
                        out=shf, in0=shf, scalar1=-1.0,
                        scalar2=float(0x5F3759DF), op0=OP.mult, op1=OP.add,
                    )
                    yb = stats.tile([P, UT], u32)
                    nc.vector.tensor_copy(yb, shf)
                    rinv = yb[:].bitcast(f32)
                    nwt = stats.tile([P, UT], f32)
                    for _ in range(newton):
                        nc.vector.tensor_mul(nwt, rinv, rinv)
                        nc.vector.tensor_mul(nwt, nwt, ms)
                        nc.vector.tensor_scalar(
                            out=nwt, in0=nwt, scalar1=-0.5, scalar2=1.5,
                            op0=OP.mult, op1=OP.add,
                        )
                        nc.vector.tensor_mul(rinv, rinv, nwt)

                    lg = stats.tile([P, UT], f32)
                    nc.vector.tensor_mul(lg, qd, rinv)
                    lg3 = lg[:].rearrange("p (u t) -> p u t", u=U)
                    mx = stats.tile([P, U], f32)
                    nc.vector.tensor_reduce(
                        out=mx, in_=lg3, axis=mybir.AxisListType.X, op=OP.max
                    )
                    mxa = mx[:, :]
                    mxb = bass.AP(tensor=mxa.tensor, offset=mxa.offset,
                                  ap=[mxa.ap[0], mxa.ap[1], [0, T_act]])
                    lgc = stats.tile([P, UT], f32, tag="lgc")
                    nc.vector.tensor_tensor(
                        out=lgc[:].rearrange("p (u t) -> p u t", u=U),
                        in0=lg3, in1=mxb, op=OP.subtract,
                    )
                    ex = stats.tile([P, UT], f32, tag="ex")
                    nc.scalar.activation(out=ex, in_=lgc, func=AF.Exp)
                    sume = stats.tile([P, U], f32)
                    nc.vector.tensor_reduce(
                        out=sume,
                        in_=ex[:].rearrange("p (u t) -> p u t", u=U),
                        axis=mybir.AxisListType.X, op=OP.add,
                    )
                    rsum = stats.tile([P, U], f32, tag="rsum")
                    nc.vector.reciprocal(rsum, sume)

                    this_po = []
                    for u in range(U):
                        dg_all = diagp.tile([P, T_act, P], f32r, tag="dg")
                        ida = id_t[:, :]
                        wa = ex[:, u * T_act : (u + 1) * T_act]
                        idb = bass.AP(tensor=ida.tensor, offset=ida.offset,
                                      ap=[ida.ap[0], [0, T_act], ida.ap[1]])
                        wb = bass.AP(tensor=wa.tensor, offset=wa.offset,
                                     ap=[wa.ap[0], wa.ap[1], [0, P]])
                        nc.vector.tensor_tensor(out=dg_all, in0=idb, in1=wb,
                                                op=OP.mult)

                        pos = []
                        for h in range(nh):
                            po = psump.tile([P, 512], f32, tag=f"po{u}{h}")
                            pos.append(po)
                        for t in range(T_act):
                            lhsT = dg_all[:, t, :]
                            for h in range(nh):
                                cs = slice(h * 512, (h + 1) * 512)
                                nc.tensor.matmul(
                                    pos[h], lhsT=lhsT, rhs=et(t, u, cs),
                                    start=(t == 0), stop=(t == T_act - 1),
                                )
                        for h in range(nh):
                            this_po.append((u, h, pos[h]))

                    if pending is not None:
                        flush(pending)
                    pending = (this_po, rsum, r0)
                flush(pending)

    nc.finalize()
    return nc


# ---------------------------------------------------------------------------
# v1 builder (previous session's kernel), kept for A/B benchmarking.
# ---------------------------------------------------------------------------


def _build_kernel(T_act, R, D, pe_dtype="f32", n_dve=0, n_gps=0, rep=1,
                  mode="full", bench_rep=False):
    f32 = mybir.dt.float32
    assert R % P == 0 and D % 512 == 0
    n_tiles = R // P
    nh = D // 512

    nc = bacc.Bacc()
    ent, qv, ident, repc, out = _declare_io(nc, T_act, R, D, bench_rep)

    with tile.TileContext(nc) as tc:
        with (
            tc.tile_pool(name="singles", bufs=1) as singles,
            tc.tile_pool(name="ebuf", bufs=2) as ebuf,
            tc.tile_pool(name="stats", bufs=2) as stats,
            tc.tile_pool(name="scr", bufs=2) as scr,
            tc.tile_pool(name="diag", bufs=3) as diagp,
            tc.tile_pool(name="outs", bufs=3) as outs,
            tc.tile_pool(name="psum", bufs=2, space="PSUM") as psump,
        ):
            qb = singles.tile([P, D], f32)
            nc.gpsimd.dma_start(out=qb, in_=qv[:, :].to_broadcast((P, D)))
            id_t = singles.tile([P, P], f32)
            nc.sync.dma_start(out=id_t, in_=ident[:, :])

            if bench_rep:
                rt = singles.tile([1, 1], mybir.dt.uint32)
                nc.sync.dma_start(out=rt, in_=repc[:, :])
                _, (repv,) = nc.values_load_multi_w_load_instructions(
                    rt[0:1, 0:1], min_val=1, max_val=1 << 20
                )
                loop_ctx = tc.For_i(0, repv, 1)
            elif rep > 1:
                loop_ctx = tc.For_i(0, rep, 1)
            else:
                loop_ctx = contextlib.nullcontext()
            with loop_ctx:
                _loop_body(
                    nc, tc, T_act, D, n_tiles, nh, pe_dtype, n_dve, n_gps,
                    ent, out, qb, id_t, ebuf, stats, scr, diagp, outs, psump,
                    mode,
                )

    nc.finalize()
    return nc


def _loop_body(
    nc, tc, T_act, D, n_tiles, nh, pe_dtype, n_dve, n_gps,
    ent, out, qb, id_t, ebuf, stats, scr, diagp, outs, psump,
    mode="full",
):
    f32 = mybir.dt.float32
    AF = mybir.ActivationFunctionType
    OP = mybir.AluOpType
    for i in range(n_tiles):
        r0 = i * P
        e = ebuf.tile([P, T_act, D], f32, tag="e")
        nc.sync.dma_start(
            out=e,
            in_=ent[:, r0 : r0 + P, :].rearrange("t s d -> s t d"),
        )

        if mode == "dmaonly":
            ob = outs.tile([P, D], f32)
            nc.scalar.copy(out=ob, in_=e[:, 0, :])
            nc.scalar.dma_start(out=out[r0 : r0 + P, :], in_=ob)
            continue

        ssq = stats.tile([P, T_act], f32)
        qd = stats.tile([P, T_act], f32)
        sq_scr = scr.tile([P, D], f32)
        qd_scr = scr.tile([P, D], f32)
        for t in range(T_act):
            nc.scalar.activation(
                out=sq_scr,
                in_=e[:, t, :],
                func=AF.Square,
                accum_out=ssq[:, t : t + 1],
            )
            nc.vector.scalar_tensor_tensor(
                out=qd_scr,
                in0=e[:, t, :],
                scalar=0.0,
                in1=qb,
                op0=OP.bypass,
                op1=OP.mult,
                accum_out=qd[:, t : t + 1],
            )

        # ms = ssq/D + eps; rinv = rsqrt(ms) via integer-seed Newton
        ms = stats.tile([P, T_act], f32)
        nc.vector.tensor_scalar(
            out=ms,
            in0=ssq,
            scalar1=1.0 / D,
            scalar2=float(NORM_EPS),
            op0=OP.mult,
            op1=OP.add,
        )
        u32 = mybir.dt.uint32
        sh = stats.tile([P, T_act], u32)
        nc.vector.tensor_scalar(
            out=sh,
            in0=ms[:].bitcast(u32),
            scalar1=1,
            scalar2=None,
            op0=OP.logical_shift_right,
        )
        shf = stats.tile([P, T_act], f32)
        nc.vector.tensor_copy(shf, sh)
        nc.vector.tensor_scalar(
            out=shf,
            in0=shf,
            scalar1=-1.0,
            scalar2=float(0x5F3759DF),
            op0=OP.mult,
            op1=OP.add,
        )
        yb = stats.tile([P, T_act], u32)
        nc.vector.tensor_copy(yb, shf)
        rinv = yb[:].bitcast(f32)
        nwt = stats.tile([P, T_act], f32)
        for _ in range(2):
            nc.vector.tensor_mul(nwt, rinv, rinv)
            nc.vector.tensor_mul(nwt, nwt, ms)
            nc.vector.tensor_scalar(
                out=nwt,
                in0=nwt,
                scalar1=-0.5,
                scalar2=1.5,
                op0=OP.mult,
                op1=OP.add,
            )
            nc.vector.tensor_mul(rinv, rinv, nwt)

        lg = stats.tile([P, T_act], f32)
        nc.vector.tensor_mul(lg, qd, rinv)

        mx = stats.tile([P, 1], f32)
        nc.vector.tensor_reduce(
            out=mx, in_=lg, axis=mybir.AxisListType.X, op=OP.max
        )
        negm = stats.tile([P, 1], f32)
        nc.vector.tensor_scalar_mul(negm, mx, -1.0)
        ex = stats.tile([P, T_act], f32)
        sume = stats.tile([P, 1], f32)
        nc.scalar.activation(
            out=ex, in_=lg, func=AF.Exp, bias=negm, accum_out=sume
        )
        rsum = stats.tile([P, 1], f32)
        nc.vector.reciprocal(rsum, sume)
        w = stats.tile([P, T_act], f32)
        nc.vector.tensor_scalar_mul(w, ex, rsum)

        _p3(nc, T_act, D, nh, pe_dtype, n_dve, n_gps, e, w, id_t,
            diagp, outs, psump, out, r0)


def _p3(nc, T_act, D, nh, pe_dtype, n_dve, n_gps, e, w, id_t, diagp, outs,
        psump, out, r0):
    f32 = mybir.dt.float32
    OP = mybir.AluOpType
    n_pe = T_act - n_dve - n_gps
    assert n_pe >= 1
    f32r = mybir.dt.float32r
    dg_all = diagp.tile([P, n_pe, P], f32, tag="dg")
    ida = id_t[:, :]
    wa = w[:, 0:n_pe]
    idb = bass.AP(tensor=ida.tensor, offset=ida.offset,
                  ap=[ida.ap[0], [0, n_pe], ida.ap[1]])
    wb = bass.AP(tensor=wa.tensor, offset=wa.offset,
                 ap=[wa.ap[0], wa.ap[1], [0, P]])
    nc.vector.tensor_tensor(out=dg_all, in0=idb, in1=wb, op=OP.mult)

    po = psump.tile([P, D], f32)
    for h in range(nh):
        cs = slice(h * 512, (h + 1) * 512)
        for t in range(n_pe):
            lhsT = dg_all[:, t, :]
            rhs = e[:, t, cs]
            if pe_dtype == "f32r":
                lhsT = lhsT.bitcast(f32r)
                rhs = rhs.bitcast(f32r)
            nc.tensor.matmul(
                po[:, cs],
                lhsT=lhsT,
                rhs=rhs,
                start=(t == 0),
                stop=(t == n_pe - 1),
            )

    ob = outs.tile([P, D], f32)
    nc.scalar.copy(out=ob, in_=po)
    for j, t in enumerate(range(n_pe, T_act)):
        eng = nc.vector if j < n_dve else nc.gpsimd
        eng.scalar_tensor_tensor(
            out=ob,
            in0=e[:, t, :],
            scalar=w[:, t : t + 1],
            in1=ob,
            op0=OP.mult,
            op1=OP.add,
        )
    nc.scalar.dma_start(out=out[r0 : r0 + P, :], in_=ob)


def _get_kernel(T_act, R, D):
    key = (T_act, R, D)
    if key not in _kernel_cache:
        if T_act == 12 and R % 256 == 0 and D == 1024:
            # tuned fast path (the graded shape): ~339 us/core on 8x trn2,
            # DMA floor for this layout is ~321 us
            _kernel_cache[key] = _build_v7(T_act, R, D, n_tg=12, ebufs=19,
                                           obufs=3, sbufs=3)
        else:
            # correctness fallback for other shapes (e.g. n_active <= 0)
            _kernel_cache[key] = _build_v2(T_act, R, D, mm_dtype="f32rd",
                                           dg_eng="vector", ebufs=2)
    return _kernel_cache[key]


def kernel(entries, proj, norm_scale, n_active, block_idx):
    entries = np.asarray(entries)
    proj = np.asarray(proj, dtype=np.float32)
    norm_scale = np.asarray(norm_scale, dtype=np.float32)
    if entries.dtype != np.float32:
        entries = entries.astype(np.float32)
    maxT, B, S, D = entries.shape
    na = int(np.asarray(n_active))
    bi = int(np.asarray(block_idx))

    if na <= 0:
        # everything masked -> softmax of equal (-1e9) logits = uniform mean
        T_act = maxT
        qprime = np.zeros((D,), dtype=np.float32)
    else:
        T_act = min(na, maxT)
        qprime = (proj[min(bi, maxT - 1)] * norm_scale).astype(np.float32)

    rows = B * S
    assert rows % (N_CORES * P) == 0, f"rows={rows} not divisible by {N_CORES * P}"
    R = rows // N_CORES

    ent_flat = entries[:T_act].reshape(T_act, rows, D)
    ident = np.eye(P, dtype=np.float32)
    qv = qprime.reshape(1, D)

    nc = _get_kernel(T_act, R, D)

    in_maps = []
    for c in range(N_CORES):
        in_maps.append(
            {
                "ent": np.ascontiguousarray(ent_flat[:, c * R : (c + 1) * R, :]),
                "qv": qv,
                "ident": ident,
            }
        )

    res = run_bass_kernel_spmd(nc, in_maps, list(range(N_CORES)))
    global _last_results
    _last_results = res
    parts = [res.results[c]["out"] for c in range(N_CORES)]
    return np.concatenate(parts, axis=0).reshape(B, S, D)


_last_results = None
